# revision 16
# baseline (speedup 1.0000x reference)
"""Trainium2 Bass kernel for nn_DeepMapping2D (histogram_binning).

Reference semantics: per cloud, quantize points to integer mm bins
(q = round_half_even(1000*p)), histogram into a 1024x1024 grid (shifted by
per-cloud coordinate minima), threshold counts (count/N > 2e-4 <=> count>=53),
sort the 0/1 occupancy descending, truncate to TOPK.  The sorted vector is K
ones then zeros, K = #bins with count >= 53.  Shifting by the minima is a
bijection on occupied bins, so K is shift-invariant and the device can work
on unshifted bin ids s = qx*1024 + qz (fine id, < 2^20).

Device algorithm (exact, two launches, all heavy counting on device):

Phase 1 (screen): per cloud, the exact 2^14-bin coarse histogram H14 over
c14 = s>>6, computed as a PSUM-matmul scatter: per column of 128 points,
build 128-wide one-hots of hi7=c14>>7 and lo7=c14&127 by comparing a
constant iota row against the point's value (DVE tensor_scalar is_equal with
a per-partition scalar), then accumulate onehot_hi^T @ onehot_lo into PSUM
(bf16 0/1 inputs are exact; fp32 accumulation).  H14, clamped to u8, goes
back to the host (1 MB total).

Host: candidate cells = {c14 : H14[c14] >= 53} (every fine bin with count
>= 53 lives in one, since H14 upper-bounds its 64 fine bins).  ~1.4k/cloud
for the rbg-generated inputs.  Padded with -1 to NCHUNK*128 int16.

Phase 2 (refine): per cloud, exact fine counts for every candidate cell:
per column, one membership one-hot against the candidate row (int16
candidates streamed at DVE 4x, compared against the point's c14 as the
per-partition scalar) and one 64-wide one-hot of low6 = s&63; NCHUNK
matmuls accumulate membership^T @ onehot_low6 into PSUM -> exact
[candidate, low6] fine counts.  Threshold >= 53, count via ones^T @ mask
matmul, giving K per cloud.  The host formats the final rows (K ones then
zeros) from the device-computed K values.

Transport optimization (the axon tunnel runs at ~35 MB/s, so bytes moved
dominate wall time): the host quantizes once into two integer planes,
c14 = s>>6 (uint16) and lo6 = s&63 (uint8) - 3 B/point = 50 MB instead of
the 8 B/point raw floats - and uploads them a single time.  Both phases
run through a jit(shard_map(bass_exec)) callable (the same primitive
bass_utils.run_bass_kernel_spmd lowers to under axon) against the SAME
device-resident plane arrays, so phase 2 re-reads them from device DRAM
instead of re-shipping 128 MB.  Quantization is pipelined per core-shard
with the uploads.

Host guards keep the kernel exact for arbitrary inputs: clouds with
coordinates outside [0, 1023] mm (or counts exceeding the candidate
capacity) fall back to an exact numpy recomputation of that cloud.

Sharding: data-parallel over batch: 64 clouds -> 8 cores x 8 clouds.
"""

import os
import numpy as np

B = 64
N = 262144
TOPK = 5120
NCORES = 8
CLOUDS_PER_CORE = B // NCORES
P = 128
F = N // P
GZ = 1024
NCHUNK = 12  # candidate capacity = NCHUNK*128 cells per cloud
CAND_CAP = NCHUNK * P
THRESH_COUNT = 53.0
C23 = 12582912.0  # 1.5 * 2^23

_cache = {}
_DEBUG = os.environ.get("KERNEL_DEBUG", "0") == "1"


def _dbg(msg, t0=None):
    if _DEBUG:
        import time

        if t0 is not None:
            print(f"[kernel] {msg}: {time.time()-t0:.3f}s", flush=True)
        else:
            print(f"[kernel] {msg}", flush=True)


def build_phase1(n_clouds=CLOUDS_PER_CORE, n_points=N, unroll=32):
    """Per-cloud exact 2^14-bin coarse histogram -> DRAM (u8, clamped)."""
    import concourse.bass as bass
    import concourse.mybir as mybir
    from concourse.tile import TileContext
    from concourse import bacc

    f32, bf16 = mybir.dt.float32, mybir.dt.bfloat16
    i32, u16, u8 = mybir.dt.int32, mybir.dt.uint16, mybir.dt.uint8
    op = mybir.AluOpType
    Fl = n_points // P

    nc = bacc.Bacc("TRN2", target_bir_lowering=False, debug=False)
    nq = n_clouds // 4
    c14ps = [
        nc.declare_dram_parameter(f"c14p{q}", [nq, n_points], u16, isOutput=False)
        for q in range(4)
    ]
    h14 = nc.declare_dram_parameter("h14", [n_clouds, P, P], u8, isOutput=True)

    with TileContext(nc) as tc:
        with (
            tc.tile_pool(name="const", bufs=1) as constp,
            tc.tile_pool(name="raw", bufs=2) as rawp,
            tc.tile_pool(name="chain", bufs=2) as chainp,
            tc.tile_pool(name="hilo", bufs=1) as hilop,
            tc.tile_pool(name="oh", bufs=8) as ohp,
            tc.tile_pool(name="hout", bufs=2) as houtp,
            tc.tile_pool(name="psum", bufs=1, space="PSUM") as psump,
        ):
            iota_i = constp.tile([P, P], i32)
            nc.gpsimd.iota(iota_i[:], pattern=[[1, P]], base=0, channel_multiplier=0)
            iota_bf = constp.tile([P, P], bf16)
            nc.vector.tensor_copy(out=iota_bf[:], in_=iota_i[:])

            this, tlos, hists = [], [], []
            for c in range(n_clouds):
                rc = rawp.tile([P, Fl], u16, tag="rc")
                src = c14ps[c // nq][c % nq]
                nc.gpsimd.dma_start(out=rc[:], in_=src.rearrange("(p f) -> p f", p=P))
                tc14 = chainp.tile([P, Fl], f32, tag="tc14")
                nc.vector.tensor_copy(out=tc14[:], in_=rc[:])
                # hi7 = floor(c14/128); lo7 = c14 - 128*hi7
                thif = chainp.tile([P, Fl], f32, tag="thif")
                nc.vector.tensor_scalar(
                    out=thif[:], in0=tc14[:], scalar1=0.0078125,
                    scalar2=0.49609375, op0=op.mult, op1=op.subtract,
                )
                thi = hilop.tile([P, Fl], f32, tag=f"thi{c}")
                nc.vector.tensor_scalar(
                    out=thi[:], in0=thif[:], scalar1=C23, scalar2=C23,
                    op0=op.add, op1=op.subtract,
                )
                tlo = hilop.tile([P, Fl], f32, tag=f"tlo{c}")
                nc.vector.scalar_tensor_tensor(
                    out=tlo[:], in0=thi[:], scalar=-128.0, in1=tc14[:],
                    op0=op.mult, op1=op.add,
                )
                this.append(thi)
                tlos.append(tlo)
                hist = psump.tile([P, P], f32, tag=f"hist{c}")
                nc.vector.memset(hist[:], 0.0)
                hists.append(hist)

            def body(iv):
                for c in range(n_clouds):
                    ohh = ohp.tile([P, P], bf16, tag="ohh")
                    ohl = ohp.tile([P, P], bf16, tag="ohl")
                    nc.vector.tensor_scalar(
                        out=ohh[:], in0=iota_bf[:],
                        scalar1=this[c][:, bass.ds(iv, 1)], scalar2=None,
                        op0=op.is_equal,
                    )
                    nc.vector.tensor_scalar(
                        out=ohl[:], in0=iota_bf[:],
                        scalar1=tlos[c][:, bass.ds(iv, 1)], scalar2=None,
                        op0=op.is_equal,
                    )
                    nc.tensor.matmul(
                        out=hists[c][:], lhsT=ohh[:], rhs=ohl[:],
                        start=False, stop=True, skip_group_check=True,
                    )

            tc.For_i_unrolled(0, Fl, 1, body, max_unroll=unroll)

            for c in range(n_clouds):
                hcl = houtp.tile([P, P], f32, tag="hcl")
                nc.vector.tensor_scalar(
                    out=hcl[:], in0=hists[c][:], scalar1=255.0, scalar2=None,
                    op0=op.min,
                )
                hu8 = houtp.tile([P, P], u8, tag="hu8")
                nc.vector.tensor_copy(out=hu8[:], in_=hcl[:])
                nc.gpsimd.dma_start(out=h14[c], in_=hu8[:])
    nc.compile()
    return nc


def build_phase2(n_clouds=CLOUDS_PER_CORE, n_points=N, nchunk=NCHUNK, unroll=16):
    """Exact [candidate,64] fine counts -> K per cloud."""
    import concourse.bass as bass
    import concourse.mybir as mybir
    from concourse.tile import TileContext
    from concourse import bacc

    f32, bf16 = mybir.dt.float32, mybir.dt.bfloat16
    i16, i32 = mybir.dt.int16, mybir.dt.int32
    u16, u8 = mybir.dt.uint16, mybir.dt.uint8
    op = mybir.AluOpType
    Fl = n_points // P
    cap = nchunk * P

    nc = bacc.Bacc("TRN2", target_bir_lowering=False, debug=False)
    nq = n_clouds // 4
    c14ps = [
        nc.declare_dram_parameter(f"c14p{q}", [nq, n_points], u16, isOutput=False)
        for q in range(4)
    ]
    lo6ps = [
        nc.declare_dram_parameter(f"lo6p{q}", [nq, n_points], u8, isOutput=False)
        for q in range(4)
    ]
    cands = nc.declare_dram_parameter("cands", [n_clouds, cap], i16, isOutput=False)
    kvals = nc.declare_dram_parameter("kvals", [1, n_clouds], f32, isOutput=True)

    with TileContext(nc) as tc:
        with (
            tc.tile_pool(name="const", bufs=1) as constp,
            tc.tile_pool(name="raw", bufs=2) as rawp,
            tc.tile_pool(name="cloud", bufs=2) as cloudp,
            tc.tile_pool(name="oh", bufs=8) as ohp,
            tc.tile_pool(name="mk", bufs=4) as mkp,
            tc.tile_pool(name="psum", bufs=1, space="PSUM") as psump,
            tc.tile_pool(name="kps", bufs=1, space="PSUM") as kpsp,
        ):
            iota64_i = constp.tile([P, 64], i32)
            nc.gpsimd.iota(iota64_i[:], pattern=[[1, 64]], base=0, channel_multiplier=0)
            iota64_bf = constp.tile([P, 64], bf16)
            nc.vector.tensor_copy(out=iota64_bf[:], in_=iota64_i[:])
            ones_bf = constp.tile([P, 1], bf16)
            nc.vector.memset(ones_bf[:], 1.0)
            kv_sb = constp.tile([1, n_clouds], f32)

            for c in range(n_clouds):
                rc = rawp.tile([P, Fl], u16, tag="rc")
                csrc = c14ps[c // nq][c % nq]
                nc.gpsimd.dma_start(out=rc[:], in_=csrc.rearrange("(p f) -> p f", p=P))
                tc14 = cloudp.tile([P, Fl], f32, tag="tc14")
                nc.vector.tensor_copy(out=tc14[:], in_=rc[:])
                rl = rawp.tile([P, Fl], u8, tag="rl")
                lsrc = lo6ps[c // nq][c % nq]
                nc.gpsimd.dma_start(out=rl[:], in_=lsrc.rearrange("(p f) -> p f", p=P))
                tlow6 = cloudp.tile([P, Fl], f32, tag="tlow6")
                nc.vector.tensor_copy(out=tlow6[:], in_=rl[:])

                # candidate row broadcast to all partitions
                candbc = cloudp.tile([P, cap], i16, tag="candbc")
                cand_src = bass.AP(
                    tensor=cands.tensor if hasattr(cands, "tensor") else cands,
                    offset=c * cap,
                    ap=[[0, P], [1, cap]],
                )
                nc.gpsimd.dma_start(out=candbc[:], in_=cand_src)

                hist = psump.tile([P, cap], f32, tag="hist")
                nc.vector.memset(hist[:], 0.0)

                def body(iv):
                    memb = ohp.tile([P, cap], bf16, tag="memb")
                    loh = ohp.tile([P, 64], bf16, tag="loh")
                    nc.vector.tensor_scalar(
                        out=memb[:], in0=candbc[:],
                        scalar1=tc14[:, bass.ds(iv, 1)], scalar2=None,
                        op0=op.is_equal,
                    )
                    nc.vector.tensor_scalar(
                        out=loh[:], in0=iota64_bf[:],
                        scalar1=tlow6[:, bass.ds(iv, 1)], scalar2=None,
                        op0=op.is_equal,
                    )
                    # transposed accumulation: hist[w, cand] += loh^T @ memb,
                    # 512-wide moving slices so the 64-wide stationary loh is
                    # shared and PE streams at full width
                    for g in range(cap // 512):
                        nc.tensor.matmul(
                            out=hist[:64, g * 512 : (g + 1) * 512],
                            lhsT=loh[:],
                            rhs=memb[:, g * 512 : (g + 1) * 512],
                            start=False, stop=True, skip_group_check=True,
                        )

                tc.For_i_unrolled(0, Fl, 1, body, max_unroll=unroll)

                # K = sum over candidates/low6 of [count >= 53]
                kps = kpsp.tile([1, cap], f32, tag="kps")
                for g in range(cap // 512):
                    mask = mkp.tile([P, 512], bf16, tag="mask")
                    nc.vector.tensor_scalar(
                        out=mask[:64, :], in0=hist[:64, g * 512 : (g + 1) * 512],
                        scalar1=52.5, scalar2=None, op0=op.is_ge,
                    )
                    nc.tensor.matmul(
                        out=kps[:1, g * 512 : (g + 1) * 512],
                        lhsT=ones_bf[:64, :], rhs=mask[:64, :],
                        start=True, stop=True,
                    )
                nc.vector.tensor_reduce(
                    out=kv_sb[:1, c : c + 1], in_=kps[:],
                    axis=mybir.AxisListType.X, op=op.add,
                )

            nc.gpsimd.dma_start(out=kvals[:, :], in_=kv_sb[:])
    nc.compile()
    return nc


class _Runner:
    """jit(shard_map(bass_exec)) callable over 8 cores with device-resident
    inputs.  Mirrors concourse.bass2jax.run_bass_via_pjrt's lowering (the
    @via_axon target of bass_utils.run_bass_kernel_spmd), but accepts jax
    Arrays already placed on the devices so repeated launches don't re-ship
    inputs, and keeps the (never-donated, fully-overwritten) output
    parameter slots device-resident too."""

    def __init__(self, nc, n_cores=NCORES):
        import jax
        from concourse import bass2jax
        import concourse.mybir as mybir
        from jax.experimental.shard_map import shard_map
        from jax.sharding import Mesh, PartitionSpec, NamedSharding

        bass2jax.install_neuronx_cc_hook()
        assert not nc.dbg_callbacks if nc.dbg_addr is not None else True
        partition_name = (
            nc.partition_id_tensor.name if nc.partition_id_tensor else None
        )
        self.jax = jax
        self.n_cores = n_cores
        devices = jax.devices()[:n_cores]
        assert len(devices) == n_cores
        self.devices = devices
        self.mesh = Mesh(np.asarray(devices), ("core",))
        self.sharding = NamedSharding(self.mesh, PartitionSpec("core"))

        in_names, out_names, out_avals = [], [], []
        in_meta = {}
        for alloc in nc.m.functions[0].allocations:
            if not isinstance(alloc, mybir.MemoryLocationSet):
                continue
            name = alloc.memorylocations[0].name
            if alloc.kind == "ExternalInput":
                if name == partition_name:
                    continue
                in_names.append(name)
                in_meta[name] = (tuple(alloc.tensor_shape), mybir.dt.np(alloc.dtype))
            elif alloc.kind == "ExternalOutput":
                out_names.append(name)
                out_avals.append(
                    jax.core.ShapedArray(
                        tuple(alloc.tensor_shape), mybir.dt.np(alloc.dtype)
                    )
                )
        self.in_names, self.out_names = in_names, out_names
        self.in_meta = in_meta
        all_in = tuple(in_names) + tuple(out_names)
        if partition_name is not None:
            all_in = all_in + (partition_name,)

        def _body(*args):
            operands = list(args)
            if partition_name is not None:
                operands.append(bass2jax.partition_id_tensor())
            outs = bass2jax._bass_exec_p.bind(
                *operands,
                out_avals=tuple(out_avals),
                in_names=all_in,
                out_names=tuple(out_names),
                lowering_input_output_aliases=(),
                sim_require_finite=True,
                sim_require_nnan=True,
                nc=nc,
            )
            return tuple(outs)

        pspec = PartitionSpec("core")
        n_args = len(in_names) + len(out_names)
        self.fn = jax.jit(
            shard_map(
                _body,
                mesh=self.mesh,
                in_specs=(pspec,) * n_args,
                out_specs=(pspec,) * len(out_names),
                check_rep=False,
            ),
            keep_unused=True,
        )
        # persistent device-resident buffers for the output parameter slots
        # (never donated; the kernels fully overwrite every output element)
        self.out_dummies = [
            jax.device_put(
                np.zeros((n_cores * av.shape[0], *av.shape[1:]), av.dtype),
                self.sharding,
            )
            for av in out_avals
        ]
        self.extra = {}

    def __call__(self, arrays):
        args = []
        for name in self.in_names:
            if name in arrays:
                args.append(arrays[name])
            else:
                if name not in self.extra:
                    shape, dt = self.in_meta[name]
                    z = np.zeros((self.n_cores * shape[0], *shape[1:]), dt)
                    self.extra[name] = self.jax.device_put(z, self.sharding)
                args.append(self.extra[name])
        outs = self.fn(*args, *self.out_dummies)
        return dict(zip(self.out_names, outs))


def _state():
    if "st" in _cache:
        return _cache["st"]
    import jax

    nc1 = build_phase1()
    nc2 = build_phase2()
    run1 = _Runner(nc1)
    run2 = _Runner(nc2)
    BQ = B // 4
    st = {
        "jax": jax,
        "run1": run1,
        "run2": run2,
        "devices": run1.devices,
        "sharding": run1.sharding,
        # persistent host work buffers (avoid first-touch page faults on the
        # timed warm call)
        "t": np.empty((BQ, N, 2), np.float32),
        "sf": np.empty((BQ, N), np.float32),
        "zi": np.empty((BQ, N), np.int16),
        "c14": np.empty((B, N), np.uint16),
        "lo6": np.empty((B, N), np.uint8),
        # c14 = floor(s/64) = 16*qx + floor(qz/64): exact in f32 (value
        # < 2^14, qz/64 on a 2^-6 grid, f32 ulp at 2^14 is 2^-9), and the
        # truncating u16 cast is the floor
        "wc14": np.asarray([16.0, 0.015625], np.float32),
        # device slot (core i, local cloud c) <-> original batch index:
        # quarter q = c//2 holds original clouds [16q, 16q+16), two per core
        "order": np.asarray(
            [
                16 * (c // 2) + 2 * i + (c % 2)
                for i in range(NCORES)
                for c in range(CLOUDS_PER_CORE)
            ]
        ),
    }
    _cache["st"] = st
    return st


def _quant_quarter(st, pcd_q, qi):
    """Quantize one batch-quarter [B/4, N, 2] into the c14/lo6 plane slices.
    Exact: q = rint(1000*p) in f32 (matches jnp.round); with s = qx*1024+qz,
    c14 = s>>6 = 16*qx + (qz>>6) and lo6 = s&63 = qz&63 (1024 = 0 mod 64)."""
    BQ = B // 4
    sl = slice(qi * BQ, (qi + 1) * BQ)
    t = st["t"]
    np.multiply(pcd_q, np.float32(1000.0), out=t)
    np.rint(t, out=t)
    mn = t.min(axis=1)
    mx = t.max(axis=1)
    good = (mn[:, 0] >= 0) & (mn[:, 1] >= 0) & (mx[:, 0] <= 1023) & (mx[:, 1] <= 1023)
    if not good.all():
        t[~good] = 0.0  # keep device indices in range; host recomputes these
    sf, zi = st["sf"], st["zi"]
    np.dot(t.reshape(-1, 2), st["wc14"], out=sf.reshape(-1))
    np.copyto(st["c14"][sl], sf, casting="unsafe")
    np.copyto(zi, t[..., 1], casting="unsafe")
    np.bitwise_and(zi, 63, out=zi)
    np.copyto(st["lo6"][sl], zi, casting="unsafe")
    return good


def _host_exact(points):
    """Exact numpy replica of the reference for one cloud. [N,2] f32 -> [TOPK]."""
    q = np.round(np.float32(1000.0) * points.astype(np.float32))
    xi = (q[:, 0] - q[:, 0].min()).astype(np.int64)
    zi = (q[:, 1] - q[:, 1].min()).astype(np.int64)
    idx = xi * GZ + zi
    counts = np.bincount(idx, minlength=1024 * GZ).astype(np.float32)
    occ = counts / np.float32(points.shape[0]) > np.float32(0.0002)
    k = min(int(occ.sum()), TOPK)
    out = np.zeros((TOPK,), np.float32)
    out[:k] = 1.0
    return out


def kernel(pcd):
    import time

    t_start = time.time()
    pcd = np.ascontiguousarray(np.asarray(pcd), dtype=np.float32)
    assert pcd.shape == (B, N, 2), pcd.shape
    st = _state()
    jax = st["jax"]
    sharding = st["sharding"]
    _dbg("state ready", t_start)

    # pipeline: quantize batch-quarter q on the (single) CPU, then issue its
    # (async) sharded uploads so the tunnel streams it while the CPU works on
    # quarter q+1.  c14 is issued before lo6 each round: phase 1 only reads
    # c14, so it can run while the lo6 tail is still in flight.
    BQ = B // 4
    goods = [None] * 4
    c14_d = [None] * 4
    lo6_d = [None] * 4
    quarters = pcd.reshape(4, BQ, N, 2)
    for qi in range(4):
        goods[qi] = _quant_quarter(st, quarters[qi], qi)
        sl = slice(qi * BQ, (qi + 1) * BQ)
        c14_d[qi] = jax.device_put(st["c14"][sl], sharding)
        lo6_d[qi] = jax.device_put(st["lo6"][sl], sharding)
    good = np.concatenate(goods)
    _dbg("quantize+upload issued", t_start)

    r1 = st["run1"]({f"c14p{q}": c14_d[q] for q in range(4)})
    h14 = np.asarray(r1["h14"]).reshape(B, P * P)  # device-slot order
    _dbg("phase1 done", t_start)

    order = st["order"]  # slot g -> original batch index
    good_slot = good[order]
    candmask = h14 >= THRESH_COUNT
    ncand = candmask.sum(1)
    ok = good_slot & (ncand <= CAND_CAP)
    cands = np.full((B, CAND_CAP), -1, np.int16)
    for g in range(B):
        if ok[g]:
            idx = np.flatnonzero(candmask[g])
            cands[g, : len(idx)] = idx.astype(np.int16)
    cands_d = jax.device_put(cands, sharding)
    _dbg("candidates ready", t_start)

    args2 = {f"c14p{q}": c14_d[q] for q in range(4)}
    args2.update({f"lo6p{q}": lo6_d[q] for q in range(4)})
    args2["cands"] = cands_d
    r2 = st["run2"](args2)
    kv = np.asarray(r2["kvals"]).reshape(B)
    _dbg("phase2 done", t_start)

    out = np.zeros((B, TOPK, 1), np.float32)
    iota = np.arange(TOPK)
    for g in range(B):
        b = int(order[g])
        if ok[g]:
            out[b, :, 0] = iota < kv[g]
        else:
            out[b, :, 0] = _host_exact(pcd[b])
    _dbg("assembled", t_start)
    return out


# revision 18
# speedup vs baseline: 1.1257x; 1.1257x over previous
"""Trainium2 Bass kernel for nn_DeepMapping2D (histogram_binning).

Reference semantics: per cloud, quantize points to integer mm bins
(q = round_half_even(1000*p)), histogram into a 1024x1024 grid (shifted by
per-cloud coordinate minima), threshold counts (count/N > 2e-4 <=> count>=53),
sort the 0/1 occupancy descending, truncate to TOPK.  The sorted vector is K
ones then zeros, K = #bins with count >= 53.  Shifting by the minima is a
bijection on occupied bins, so K is shift-invariant and the device can work
on unshifted bin ids s = qx*1024 + qz (fine id, < 2^20).

Device algorithm (exact, two launches, all heavy counting on device):

Phase 1 (screen): per cloud, the exact 2^14-bin coarse histogram H14 over
c14 = s>>6, computed as a PSUM-matmul scatter: per column of 128 points,
build 128-wide one-hots of hi7=c14>>7 and lo7=c14&127 by comparing a
constant iota row against the point's value (DVE tensor_scalar is_equal with
a per-partition scalar), then accumulate onehot_hi^T @ onehot_lo into PSUM
(bf16 0/1 inputs are exact; fp32 accumulation).  H14, clamped to u8, goes
back to the host (1 MB total).

Host: candidate cells = {c14 : H14[c14] >= 53} (every fine bin with count
>= 53 lives in one, since H14 upper-bounds its 64 fine bins).  ~1.4k/cloud
for the rbg-generated inputs.  Padded with -1 to NCHUNK*128 int16.

Phase 2 (refine): per cloud, exact fine counts for every candidate cell:
per column, one membership one-hot against the candidate row (int16
candidates streamed at DVE 4x, compared against the point's c14 as the
per-partition scalar) and one 64-wide one-hot of low6 = s&63; NCHUNK
matmuls accumulate membership^T @ onehot_low6 into PSUM -> exact
[candidate, low6] fine counts.  Threshold >= 53, count via ones^T @ mask
matmul, giving K per cloud.  The host formats the final rows (K ones then
zeros) from the device-computed K values.

Transport optimization (the axon tunnel runs at ~35 MB/s, so bytes moved
dominate wall time): the host quantizes once into two integer planes,
c14 = s>>6 (uint16) and lo6 = s&63 (uint8) - 3 B/point = 50 MB instead of
the 8 B/point raw floats - and uploads them a single time.  Both phases
run through a jit(shard_map(bass_exec)) callable (the same primitive
bass_utils.run_bass_kernel_spmd lowers to under axon) against the SAME
device-resident plane arrays, so phase 2 re-reads them from device DRAM
instead of re-shipping 128 MB.  Quantization is pipelined per core-shard
with the uploads.

Host guards keep the kernel exact for arbitrary inputs: clouds with
coordinates outside [0, 1023] mm (or counts exceeding the candidate
capacity) fall back to an exact numpy recomputation of that cloud.

Sharding: data-parallel over batch: 64 clouds -> 8 cores x 8 clouds.
"""

import os
import numpy as np

B = 64
N = 262144
TOPK = 5120
NCORES = 8
CLOUDS_PER_CORE = B // NCORES
P = 128
F = N // P
GZ = 1024
NCHUNK = 12  # candidate capacity = NCHUNK*128 cells per cloud
CAND_CAP = NCHUNK * P
THRESH_COUNT = 53.0
C23 = 12582912.0  # 1.5 * 2^23

_cache = {}
_DEBUG = os.environ.get("KERNEL_DEBUG", "0") == "1"


def _dbg(msg, t0=None):
    if _DEBUG:
        import time

        if t0 is not None:
            print(f"[kernel] {msg}: {time.time()-t0:.3f}s", flush=True)
        else:
            print(f"[kernel] {msg}", flush=True)


def build_phase1(n_clouds=CLOUDS_PER_CORE, n_points=N, unroll=32):
    """Per-cloud exact 2^14-bin coarse histogram -> DRAM (u8, clamped)."""
    import concourse.bass as bass
    import concourse.mybir as mybir
    from concourse.tile import TileContext
    from concourse import bacc

    f32, bf16 = mybir.dt.float32, mybir.dt.bfloat16
    i32, u16, u8 = mybir.dt.int32, mybir.dt.uint16, mybir.dt.uint8
    op = mybir.AluOpType
    Fl = n_points // P

    nc = bacc.Bacc("TRN2", target_bir_lowering=False, debug=False)
    nq = n_clouds // 4
    c14ps = [
        nc.declare_dram_parameter(f"c14p{q}", [nq, n_points], u16, isOutput=False)
        for q in range(4)
    ]
    h14 = nc.declare_dram_parameter("h14", [n_clouds, P, P], u8, isOutput=True)

    with TileContext(nc) as tc:
        with (
            tc.tile_pool(name="const", bufs=1) as constp,
            tc.tile_pool(name="raw", bufs=2) as rawp,
            tc.tile_pool(name="chain", bufs=2) as chainp,
            tc.tile_pool(name="hilo", bufs=1) as hilop,
            tc.tile_pool(name="oh", bufs=8) as ohp,
            tc.tile_pool(name="hout", bufs=2) as houtp,
            tc.tile_pool(name="psum", bufs=1, space="PSUM") as psump,
        ):
            iota_i = constp.tile([P, P], i32)
            nc.gpsimd.iota(iota_i[:], pattern=[[1, P]], base=0, channel_multiplier=0)
            iota_bf = constp.tile([P, P], bf16)
            nc.vector.tensor_copy(out=iota_bf[:], in_=iota_i[:])

            this, tlos, hists = [], [], []
            for c in range(n_clouds):
                rc = rawp.tile([P, Fl], u16, tag="rc")
                src = c14ps[c // nq][c % nq]
                nc.gpsimd.dma_start(out=rc[:], in_=src.rearrange("(p f) -> p f", p=P))
                tc14 = chainp.tile([P, Fl], f32, tag="tc14")
                nc.vector.tensor_copy(out=tc14[:], in_=rc[:])
                # hi7 = floor(c14/128); lo7 = c14 - 128*hi7
                thif = chainp.tile([P, Fl], f32, tag="thif")
                nc.vector.tensor_scalar(
                    out=thif[:], in0=tc14[:], scalar1=0.0078125,
                    scalar2=0.49609375, op0=op.mult, op1=op.subtract,
                )
                thi = hilop.tile([P, Fl], f32, tag=f"thi{c}")
                nc.vector.tensor_scalar(
                    out=thi[:], in0=thif[:], scalar1=C23, scalar2=C23,
                    op0=op.add, op1=op.subtract,
                )
                tlo = hilop.tile([P, Fl], f32, tag=f"tlo{c}")
                nc.vector.scalar_tensor_tensor(
                    out=tlo[:], in0=thi[:], scalar=-128.0, in1=tc14[:],
                    op0=op.mult, op1=op.add,
                )
                this.append(thi)
                tlos.append(tlo)
                hist = psump.tile([P, P], f32, tag=f"hist{c}")
                nc.vector.memset(hist[:], 0.0)
                hists.append(hist)

            def body(iv):
                for c in range(n_clouds):
                    ohh = ohp.tile([P, P], bf16, tag="ohh")
                    ohl = ohp.tile([P, P], bf16, tag="ohl")
                    nc.vector.tensor_scalar(
                        out=ohh[:], in0=iota_bf[:],
                        scalar1=this[c][:, bass.ds(iv, 1)], scalar2=None,
                        op0=op.is_equal,
                    )
                    nc.vector.tensor_scalar(
                        out=ohl[:], in0=iota_bf[:],
                        scalar1=tlos[c][:, bass.ds(iv, 1)], scalar2=None,
                        op0=op.is_equal,
                    )
                    nc.tensor.matmul(
                        out=hists[c][:], lhsT=ohh[:], rhs=ohl[:],
                        start=False, stop=True, skip_group_check=True,
                    )

            tc.For_i_unrolled(0, Fl, 1, body, max_unroll=unroll)

            for c in range(n_clouds):
                hcl = houtp.tile([P, P], f32, tag="hcl")
                nc.vector.tensor_scalar(
                    out=hcl[:], in0=hists[c][:], scalar1=255.0, scalar2=None,
                    op0=op.min,
                )
                hu8 = houtp.tile([P, P], u8, tag="hu8")
                nc.vector.tensor_copy(out=hu8[:], in_=hcl[:])
                nc.gpsimd.dma_start(out=h14[c], in_=hu8[:])
    nc.compile()
    return nc


def build_phase2(n_clouds=CLOUDS_PER_CORE, n_points=N, nchunk=NCHUNK, unroll=16):
    """Exact [candidate,64] fine counts -> K per cloud."""
    import concourse.bass as bass
    import concourse.mybir as mybir
    from concourse.tile import TileContext
    from concourse import bacc

    f32, bf16 = mybir.dt.float32, mybir.dt.bfloat16
    i16, i32 = mybir.dt.int16, mybir.dt.int32
    u16, u8 = mybir.dt.uint16, mybir.dt.uint8
    op = mybir.AluOpType
    Fl = n_points // P
    cap = nchunk * P

    nc = bacc.Bacc("TRN2", target_bir_lowering=False, debug=False)
    nq = n_clouds // 4
    c14ps = [
        nc.declare_dram_parameter(f"c14p{q}", [nq, n_points], u16, isOutput=False)
        for q in range(4)
    ]
    lo6ps = [
        nc.declare_dram_parameter(f"lo6p{q}", [nq, n_points], u8, isOutput=False)
        for q in range(4)
    ]
    cands = nc.declare_dram_parameter("cands", [n_clouds, cap], i16, isOutput=False)
    kvals = nc.declare_dram_parameter("kvals", [1, n_clouds], f32, isOutput=True)

    with TileContext(nc) as tc:
        with (
            tc.tile_pool(name="const", bufs=1) as constp,
            tc.tile_pool(name="raw", bufs=2) as rawp,
            tc.tile_pool(name="cloud", bufs=2) as cloudp,
            tc.tile_pool(name="oh", bufs=8) as ohp,
            tc.tile_pool(name="mk", bufs=4) as mkp,
            tc.tile_pool(name="psum", bufs=1, space="PSUM") as psump,
            tc.tile_pool(name="kps", bufs=1, space="PSUM") as kpsp,
        ):
            iota64_i = constp.tile([P, 64], i32)
            nc.gpsimd.iota(iota64_i[:], pattern=[[1, 64]], base=0, channel_multiplier=0)
            iota64_bf = constp.tile([P, 64], bf16)
            nc.vector.tensor_copy(out=iota64_bf[:], in_=iota64_i[:])
            ones_bf = constp.tile([P, 1], bf16)
            nc.vector.memset(ones_bf[:], 1.0)
            kv_sb = constp.tile([1, n_clouds], f32)

            for c in range(n_clouds):
                rc = rawp.tile([P, Fl], u16, tag="rc")
                csrc = c14ps[c // nq][c % nq]
                nc.gpsimd.dma_start(out=rc[:], in_=csrc.rearrange("(p f) -> p f", p=P))
                tc14 = cloudp.tile([P, Fl], f32, tag="tc14")
                nc.vector.tensor_copy(out=tc14[:], in_=rc[:])
                rl = rawp.tile([P, Fl], u8, tag="rl")
                lsrc = lo6ps[c // nq][c % nq]
                nc.gpsimd.dma_start(out=rl[:], in_=lsrc.rearrange("(p f) -> p f", p=P))
                tlow6 = cloudp.tile([P, Fl], f32, tag="tlow6")
                nc.vector.tensor_copy(out=tlow6[:], in_=rl[:])

                # candidate row broadcast to all partitions
                candbc = cloudp.tile([P, cap], i16, tag="candbc")
                cand_src = bass.AP(
                    tensor=cands.tensor if hasattr(cands, "tensor") else cands,
                    offset=c * cap,
                    ap=[[0, P], [1, cap]],
                )
                nc.gpsimd.dma_start(out=candbc[:], in_=cand_src)

                hist = psump.tile([P, cap], f32, tag="hist")
                nc.vector.memset(hist[:], 0.0)

                def body(iv):
                    memb = ohp.tile([P, cap], bf16, tag="memb")
                    loh = ohp.tile([P, 64], bf16, tag="loh")
                    nc.vector.tensor_scalar(
                        out=memb[:], in0=candbc[:],
                        scalar1=tc14[:, bass.ds(iv, 1)], scalar2=None,
                        op0=op.is_equal,
                    )
                    nc.vector.tensor_scalar(
                        out=loh[:], in0=iota64_bf[:],
                        scalar1=tlow6[:, bass.ds(iv, 1)], scalar2=None,
                        op0=op.is_equal,
                    )
                    # transposed accumulation: hist[w, cand] += loh^T @ memb,
                    # 512-wide moving slices so the 64-wide stationary loh is
                    # shared and PE streams at full width
                    for g in range(cap // 512):
                        nc.tensor.matmul(
                            out=hist[:64, g * 512 : (g + 1) * 512],
                            lhsT=loh[:],
                            rhs=memb[:, g * 512 : (g + 1) * 512],
                            start=False, stop=True, skip_group_check=True,
                        )

                tc.For_i_unrolled(0, Fl, 1, body, max_unroll=unroll)

                # K = sum over candidates/low6 of [count >= 53]
                kps = kpsp.tile([1, cap], f32, tag="kps")
                for g in range(cap // 512):
                    mask = mkp.tile([P, 512], bf16, tag="mask")
                    nc.vector.tensor_scalar(
                        out=mask[:64, :], in0=hist[:64, g * 512 : (g + 1) * 512],
                        scalar1=52.5, scalar2=None, op0=op.is_ge,
                    )
                    nc.tensor.matmul(
                        out=kps[:1, g * 512 : (g + 1) * 512],
                        lhsT=ones_bf[:64, :], rhs=mask[:64, :],
                        start=True, stop=True,
                    )
                nc.vector.tensor_reduce(
                    out=kv_sb[:1, c : c + 1], in_=kps[:],
                    axis=mybir.AxisListType.X, op=op.add,
                )

            nc.gpsimd.dma_start(out=kvals[:, :], in_=kv_sb[:])
    nc.compile()
    return nc


class _Runner:
    """jit(shard_map(bass_exec)) callable over 8 cores with device-resident
    inputs.  Mirrors concourse.bass2jax.run_bass_via_pjrt's lowering (the
    @via_axon target of bass_utils.run_bass_kernel_spmd), but accepts jax
    Arrays already placed on the devices so repeated launches don't re-ship
    inputs, and keeps the (never-donated, fully-overwritten) output
    parameter slots device-resident too."""

    def __init__(self, nc, n_cores=NCORES):
        import jax
        from concourse import bass2jax
        import concourse.mybir as mybir
        from jax.experimental.shard_map import shard_map
        from jax.sharding import Mesh, PartitionSpec, NamedSharding

        bass2jax.install_neuronx_cc_hook()
        assert not nc.dbg_callbacks if nc.dbg_addr is not None else True
        partition_name = (
            nc.partition_id_tensor.name if nc.partition_id_tensor else None
        )
        self.jax = jax
        self.n_cores = n_cores
        devices = jax.devices()[:n_cores]
        assert len(devices) == n_cores
        self.devices = devices
        self.mesh = Mesh(np.asarray(devices), ("core",))
        self.sharding = NamedSharding(self.mesh, PartitionSpec("core"))

        in_names, out_names, out_avals = [], [], []
        in_meta = {}
        for alloc in nc.m.functions[0].allocations:
            if not isinstance(alloc, mybir.MemoryLocationSet):
                continue
            name = alloc.memorylocations[0].name
            if alloc.kind == "ExternalInput":
                if name == partition_name:
                    continue
                in_names.append(name)
                in_meta[name] = (tuple(alloc.tensor_shape), mybir.dt.np(alloc.dtype))
            elif alloc.kind == "ExternalOutput":
                out_names.append(name)
                out_avals.append(
                    jax.core.ShapedArray(
                        tuple(alloc.tensor_shape), mybir.dt.np(alloc.dtype)
                    )
                )
        self.in_names, self.out_names = in_names, out_names
        self.in_meta = in_meta
        all_in = tuple(in_names) + tuple(out_names)
        if partition_name is not None:
            all_in = all_in + (partition_name,)

        def _body(*args):
            operands = list(args)
            if partition_name is not None:
                operands.append(bass2jax.partition_id_tensor())
            outs = bass2jax._bass_exec_p.bind(
                *operands,
                out_avals=tuple(out_avals),
                in_names=all_in,
                out_names=tuple(out_names),
                lowering_input_output_aliases=(),
                sim_require_finite=True,
                sim_require_nnan=True,
                nc=nc,
            )
            return tuple(outs)

        pspec = PartitionSpec("core")
        n_args = len(in_names) + len(out_names)
        self.fn = jax.jit(
            shard_map(
                _body,
                mesh=self.mesh,
                in_specs=(pspec,) * n_args,
                out_specs=(pspec,) * len(out_names),
                check_rep=False,
            ),
            keep_unused=True,
        )
        # persistent device-resident buffers for the output parameter slots
        # (never donated; the kernels fully overwrite every output element)
        self.out_dummies = [
            jax.device_put(
                np.zeros((n_cores * av.shape[0], *av.shape[1:]), av.dtype),
                self.sharding,
            )
            for av in out_avals
        ]
        self.extra = {}

    def __call__(self, arrays):
        args = []
        for name in self.in_names:
            if name in arrays:
                args.append(arrays[name])
            else:
                if name not in self.extra:
                    shape, dt = self.in_meta[name]
                    z = np.zeros((self.n_cores * shape[0], *shape[1:]), dt)
                    self.extra[name] = self.jax.device_put(z, self.sharding)
                args.append(self.extra[name])
        outs = self.fn(*args, *self.out_dummies)
        return dict(zip(self.out_names, outs))


def _state():
    if "st" in _cache:
        return _cache["st"]
    import jax

    nc1 = build_phase1()
    nc2 = build_phase2()
    run1 = _Runner(nc1)
    run2 = _Runner(nc2)
    BQ = B // 4
    st = {
        "jax": jax,
        "run1": run1,
        "run2": run2,
        "devices": run1.devices,
        "sharding": run1.sharding,
        # persistent host work buffers (avoid first-touch page faults on the
        # timed warm call)
        "t": np.empty((BQ, N, 2), np.float32),
        "sf": np.empty((BQ, N), np.float32),
        "zi": np.empty((BQ, N), np.int16),
        "c14": np.empty((B, N), np.uint16),
        "lo6": np.empty((B, N), np.uint8),
        # c14 = floor(s/64) = 16*qx + floor(qz/64): exact in f32 (value
        # < 2^14, qz/64 on a 2^-6 grid, f32 ulp at 2^14 is 2^-9), and the
        # truncating u16 cast is the floor
        "wc14": np.asarray([16.0, 0.015625], np.float32),
        # device slot (core i, local cloud c) <-> original batch index:
        # quarter q = c//2 holds original clouds [16q, 16q+16), two per core
        "order": np.asarray(
            [
                16 * (c // 2) + 2 * i + (c % 2)
                for i in range(NCORES)
                for c in range(CLOUDS_PER_CORE)
            ]
        ),
    }
    _cache["st"] = st
    return st


def _quant_quarter(st, pcd_q, qi):
    """Quantize one batch-quarter [B/4, N, 2] into the c14/lo6 plane slices.
    Exact: q = rint(1000*p) in f32 (matches jnp.round); with s = qx*1024+qz,
    c14 = s>>6 = 16*qx + (qz>>6) and lo6 = s&63 = qz&63 (1024 = 0 mod 64)."""
    BQ = B // 4
    sl = slice(qi * BQ, (qi + 1) * BQ)
    t = st["t"]
    np.multiply(pcd_q, np.float32(1000.0), out=t)
    np.rint(t, out=t)
    mn = t.min(axis=1)
    mx = t.max(axis=1)
    good = (mn[:, 0] >= 0) & (mn[:, 1] >= 0) & (mx[:, 0] <= 1023) & (mx[:, 1] <= 1023)
    if not good.all():
        t[~good] = 0.0  # keep device indices in range; host recomputes these
    sf, zi = st["sf"], st["zi"]
    np.dot(t.reshape(-1, 2), st["wc14"], out=sf.reshape(-1))
    np.copyto(st["c14"][sl], sf, casting="unsafe")
    np.copyto(zi, t[..., 1], casting="unsafe")
    np.bitwise_and(zi, 63, out=zi)
    np.copyto(st["lo6"][sl], zi, casting="unsafe")
    return good


def _host_exact(points):
    """Exact numpy replica of the reference for one cloud. [N,2] f32 -> [TOPK]."""
    q = np.round(np.float32(1000.0) * points.astype(np.float32))
    xi = (q[:, 0] - q[:, 0].min()).astype(np.int64)
    zi = (q[:, 1] - q[:, 1].min()).astype(np.int64)
    idx = xi * GZ + zi
    counts = np.bincount(idx, minlength=1024 * GZ).astype(np.float32)
    occ = counts / np.float32(points.shape[0]) > np.float32(0.0002)
    k = min(int(occ.sum()), TOPK)
    out = np.zeros((TOPK,), np.float32)
    out[:k] = 1.0
    return out


def kernel(pcd):
    import time

    t_start = time.time()
    pcd = np.ascontiguousarray(np.asarray(pcd), dtype=np.float32)
    assert pcd.shape == (B, N, 2), pcd.shape
    st = _state()
    jax = st["jax"]
    sharding = st["sharding"]
    _dbg("state ready", t_start)

    # pipeline: quantize batch-quarter q on the (single) CPU while a single
    # uploader thread streams finished quarters over the tunnel (sharded
    # device_put blocks until the transfer lands, so it must live off the
    # main thread; one uploader keeps transfers big and in order).  c14 is
    # queued before lo6 each round: phase 1 only reads c14, so it can run
    # while the lo6 tail is still in flight.
    from concurrent.futures import ThreadPoolExecutor

    BQ = B // 4
    goods = [None] * 4
    quarters = pcd.reshape(4, BQ, N, 2)
    with ThreadPoolExecutor(max_workers=1) as ex:
        futs_c = [None] * 4
        futs_l = [None] * 4
        for qi in range(4):
            goods[qi] = _quant_quarter(st, quarters[qi], qi)
            sl = slice(qi * BQ, (qi + 1) * BQ)
            futs_c[qi] = ex.submit(jax.device_put, st["c14"][sl], sharding)
            futs_l[qi] = ex.submit(jax.device_put, st["lo6"][sl], sharding)
        good = np.concatenate(goods)
        _dbg("quantize done, uploads in flight", t_start)
        c14_d = [f.result() for f in futs_c]
        _dbg("c14 uploaded", t_start)

        # phase 1 runs while the lo6 tail is still streaming
        r1 = st["run1"]({f"c14p{q}": c14_d[q] for q in range(4)})
        h14 = np.asarray(r1["h14"]).reshape(B, P * P)  # device-slot order
        _dbg("phase1 done", t_start)

        order = st["order"]  # slot g -> original batch index
        good_slot = good[order]
        candmask = h14 >= THRESH_COUNT
        ncand = candmask.sum(1)
        ok = good_slot & (ncand <= CAND_CAP)
        cands = np.full((B, CAND_CAP), -1, np.int16)
        for g in range(B):
            if ok[g]:
                idx = np.flatnonzero(candmask[g])
                cands[g, : len(idx)] = idx.astype(np.int16)
        cands_d = jax.device_put(cands, sharding)
        lo6_d = [f.result() for f in futs_l]
    _dbg("candidates + lo6 ready", t_start)

    args2 = {f"c14p{q}": c14_d[q] for q in range(4)}
    args2.update({f"lo6p{q}": lo6_d[q] for q in range(4)})
    args2["cands"] = cands_d
    r2 = st["run2"](args2)
    kv = np.asarray(r2["kvals"]).reshape(B)
    _dbg("phase2 done", t_start)

    out = np.zeros((B, TOPK, 1), np.float32)
    iota = np.arange(TOPK)
    for g in range(B):
        b = int(order[g])
        if ok[g]:
            out[b, :, 0] = iota < kv[g]
        else:
            out[b, :, 0] = _host_exact(pcd[b])
    _dbg("assembled", t_start)
    return out


# revision 32
# speedup vs baseline: 1.1732x; 1.0422x over previous
"""Trainium2 Bass kernel for nn_DeepMapping2D (histogram_binning).

Reference semantics: per cloud, quantize points to integer mm bins
(q = round_half_even(1000*p)), histogram into a 1024x1024 grid (shifted by
per-cloud coordinate minima), threshold counts (count/N > 2e-4 <=> count>=53),
sort the 0/1 occupancy descending, truncate to TOPK.  The sorted vector is K
ones then zeros, K = #bins with count >= 53.  Shifting by the minima is a
bijection on occupied bins, so K is shift-invariant and the device can work
on unshifted bin ids s = qx*1024 + qz (fine id, < 2^20).

Device algorithm (exact, two launches, all heavy counting on device):

Phase 1 (screen): per cloud, the exact 2^14-bin coarse histogram H14 over
c14 = s>>6, computed as a PSUM-matmul scatter: per column of 128 points,
build 128-wide one-hots of hi7=c14>>7 and lo7=c14&127 by comparing a
constant iota row against the point's value (DVE tensor_scalar is_equal with
a per-partition scalar), then accumulate onehot_hi^T @ onehot_lo into PSUM
(bf16 0/1 inputs are exact; fp32 accumulation).  H14, clamped to u8, goes
back to the host (1 MB total).

Host: candidate cells = {c14 : H14[c14] >= 53} (every fine bin with count
>= 53 lives in one, since H14 upper-bounds its 64 fine bins).  ~1.4k/cloud
for the rbg-generated inputs.  Padded with -1 to NCHUNK*128 int16.

Phase 2 (refine): per cloud, exact fine counts for every candidate cell:
per column, one membership one-hot against the candidate row (int16
candidates streamed at DVE 4x, compared against the point's c14 as the
per-partition scalar) and one 64-wide one-hot of low6 = s&63; NCHUNK
matmuls accumulate membership^T @ onehot_low6 into PSUM -> exact
[candidate, low6] fine counts.  Threshold >= 53, count via ones^T @ mask
matmul, giving K per cloud.  The host formats the final rows (K ones then
zeros) from the device-computed K values.

Transport optimization (the axon tunnel runs at ~35 MB/s, so bytes moved
dominate wall time): the host quantizes once into two integer planes,
c14 = s>>6 (uint16) and lo6 = s&63 (uint8) - 3 B/point = 50 MB instead of
the 8 B/point raw floats - and uploads them a single time.  Both phases
run through a jit(shard_map(bass_exec)) callable (the same primitive
bass_utils.run_bass_kernel_spmd lowers to under axon) against the SAME
device-resident plane arrays, so phase 2 re-reads them from device DRAM
instead of re-shipping 128 MB.  Quantization is pipelined per core-shard
with the uploads.

Host guards keep the kernel exact for arbitrary inputs: clouds with
coordinates outside [0, 1023] mm (or counts exceeding the candidate
capacity) fall back to an exact numpy recomputation of that cloud.

Sharding: data-parallel over batch: 64 clouds -> 8 cores x 8 clouds.
"""

import os
import numpy as np

B = 64
N = 262144
TOPK = 5120
NCORES = 8
CLOUDS_PER_CORE = B // NCORES
P = 128
F = N // P
GZ = 1024
NCHUNK = 12  # candidate capacity = NCHUNK*128 cells per cloud
CAND_CAP = NCHUNK * P
THRESH_COUNT = 53.0
C23 = 12582912.0  # 1.5 * 2^23

_cache = {}
_DEBUG = os.environ.get("KERNEL_DEBUG", "0") == "1"


def _dbg(msg, t0=None):
    if _DEBUG:
        import time

        if t0 is not None:
            print(f"[kernel] {msg}: {time.time()-t0:.3f}s", flush=True)
        else:
            print(f"[kernel] {msg}", flush=True)


def build_phase1(n_clouds=CLOUDS_PER_CORE, n_points=N, unroll=32):
    """Per-cloud exact 2^14-bin coarse histogram -> DRAM (u8, clamped)."""
    import concourse.bass as bass
    import concourse.mybir as mybir
    from concourse.tile import TileContext
    from concourse import bacc

    f32, bf16 = mybir.dt.float32, mybir.dt.bfloat16
    i32, u16, u8 = mybir.dt.int32, mybir.dt.uint16, mybir.dt.uint8
    op = mybir.AluOpType
    Fl = n_points // P

    nc = bacc.Bacc("TRN2", target_bir_lowering=False, debug=False)
    nq = n_clouds // 4
    c14ps = [
        nc.declare_dram_parameter(f"c14p{q}", [nq, n_points], u16, isOutput=False)
        for q in range(4)
    ]
    h14 = nc.declare_dram_parameter("h14", [n_clouds, P, P], u8, isOutput=True)

    with TileContext(nc) as tc:
        with (
            tc.tile_pool(name="const", bufs=1) as constp,
            tc.tile_pool(name="raw", bufs=2) as rawp,
            tc.tile_pool(name="chain", bufs=2) as chainp,
            tc.tile_pool(name="hilo", bufs=1) as hilop,
            tc.tile_pool(name="oh", bufs=8) as ohp,
            tc.tile_pool(name="hout", bufs=2) as houtp,
            tc.tile_pool(name="psum", bufs=1, space="PSUM") as psump,
        ):
            iota_i = constp.tile([P, P], i32)
            nc.gpsimd.iota(iota_i[:], pattern=[[1, P]], base=0, channel_multiplier=0)
            iota_bf = constp.tile([P, P], bf16)
            nc.vector.tensor_copy(out=iota_bf[:], in_=iota_i[:])

            this, tlos, hists = [], [], []
            for c in range(n_clouds):
                rc = rawp.tile([P, Fl], u16, tag="rc")
                src = c14ps[c // nq][c % nq]
                nc.gpsimd.dma_start(out=rc[:], in_=src.rearrange("(p f) -> p f", p=P))
                tc14 = chainp.tile([P, Fl], f32, tag="tc14")
                nc.vector.tensor_copy(out=tc14[:], in_=rc[:])
                # hi7 = floor(c14/128); lo7 = c14 - 128*hi7
                thif = chainp.tile([P, Fl], f32, tag="thif")
                nc.vector.tensor_scalar(
                    out=thif[:], in0=tc14[:], scalar1=0.0078125,
                    scalar2=0.49609375, op0=op.mult, op1=op.subtract,
                )
                thi = hilop.tile([P, Fl], f32, tag=f"thi{c}")
                nc.vector.tensor_scalar(
                    out=thi[:], in0=thif[:], scalar1=C23, scalar2=C23,
                    op0=op.add, op1=op.subtract,
                )
                tlo = hilop.tile([P, Fl], f32, tag=f"tlo{c}")
                nc.vector.scalar_tensor_tensor(
                    out=tlo[:], in0=thi[:], scalar=-128.0, in1=tc14[:],
                    op0=op.mult, op1=op.add,
                )
                this.append(thi)
                tlos.append(tlo)
                hist = psump.tile([P, P], f32, tag=f"hist{c}")
                nc.vector.memset(hist[:], 0.0)
                hists.append(hist)

            def body(iv):
                for c in range(n_clouds):
                    ohh = ohp.tile([P, P], bf16, tag="ohh")
                    ohl = ohp.tile([P, P], bf16, tag="ohl")
                    nc.vector.tensor_scalar(
                        out=ohh[:], in0=iota_bf[:],
                        scalar1=this[c][:, bass.ds(iv, 1)], scalar2=None,
                        op0=op.is_equal,
                    )
                    nc.vector.tensor_scalar(
                        out=ohl[:], in0=iota_bf[:],
                        scalar1=tlos[c][:, bass.ds(iv, 1)], scalar2=None,
                        op0=op.is_equal,
                    )
                    nc.tensor.matmul(
                        out=hists[c][:], lhsT=ohh[:], rhs=ohl[:],
                        start=False, stop=True, skip_group_check=True,
                    )

            tc.For_i_unrolled(0, Fl, 1, body, max_unroll=unroll)

            for c in range(n_clouds):
                hcl = houtp.tile([P, P], f32, tag="hcl")
                nc.vector.tensor_scalar(
                    out=hcl[:], in0=hists[c][:], scalar1=255.0, scalar2=None,
                    op0=op.min,
                )
                hu8 = houtp.tile([P, P], u8, tag="hu8")
                nc.vector.tensor_copy(out=hu8[:], in_=hcl[:])
                nc.gpsimd.dma_start(out=h14[c], in_=hu8[:])
    nc.compile()
    return nc


def build_phase2(n_clouds=CLOUDS_PER_CORE, n_points=N, nchunk=NCHUNK, unroll=16):
    """Exact [candidate,64] fine counts -> K per cloud."""
    import concourse.bass as bass
    import concourse.mybir as mybir
    from concourse.tile import TileContext
    from concourse import bacc

    f32, bf16 = mybir.dt.float32, mybir.dt.bfloat16
    i16, i32 = mybir.dt.int16, mybir.dt.int32
    u16, u8 = mybir.dt.uint16, mybir.dt.uint8
    op = mybir.AluOpType
    Fl = n_points // P
    cap = nchunk * P

    nc = bacc.Bacc("TRN2", target_bir_lowering=False, debug=False)
    nq = n_clouds // 4
    c14ps = [
        nc.declare_dram_parameter(f"c14p{q}", [nq, n_points], u16, isOutput=False)
        for q in range(4)
    ]
    lo6ps = [
        nc.declare_dram_parameter(f"lo6p{q}", [nq, n_points], u8, isOutput=False)
        for q in range(4)
    ]
    cands = nc.declare_dram_parameter("cands", [n_clouds, cap], i16, isOutput=False)
    kvals = nc.declare_dram_parameter("kvals", [1, n_clouds], f32, isOutput=True)

    with TileContext(nc) as tc:
        with (
            tc.tile_pool(name="const", bufs=1) as constp,
            tc.tile_pool(name="raw", bufs=2) as rawp,
            tc.tile_pool(name="cloud", bufs=2) as cloudp,
            tc.tile_pool(name="oh", bufs=8) as ohp,
            tc.tile_pool(name="mk", bufs=4) as mkp,
            tc.tile_pool(name="psum", bufs=1, space="PSUM") as psump,
            tc.tile_pool(name="kps", bufs=1, space="PSUM") as kpsp,
        ):
            iota64_i = constp.tile([P, 64], i32)
            nc.gpsimd.iota(iota64_i[:], pattern=[[1, 64]], base=0, channel_multiplier=0)
            iota64_bf = constp.tile([P, 64], bf16)
            nc.vector.tensor_copy(out=iota64_bf[:], in_=iota64_i[:])
            ones_bf = constp.tile([P, 1], bf16)
            nc.vector.memset(ones_bf[:], 1.0)
            kv_sb = constp.tile([1, n_clouds], f32)

            for c in range(n_clouds):
                rc = rawp.tile([P, Fl], u16, tag="rc")
                csrc = c14ps[c // nq][c % nq]
                nc.gpsimd.dma_start(out=rc[:], in_=csrc.rearrange("(p f) -> p f", p=P))
                tc14 = cloudp.tile([P, Fl], f32, tag="tc14")
                nc.vector.tensor_copy(out=tc14[:], in_=rc[:])
                rl = rawp.tile([P, Fl], u8, tag="rl")
                lsrc = lo6ps[c // nq][c % nq]
                nc.gpsimd.dma_start(out=rl[:], in_=lsrc.rearrange("(p f) -> p f", p=P))
                tlow6 = cloudp.tile([P, Fl], f32, tag="tlow6")
                nc.vector.tensor_copy(out=tlow6[:], in_=rl[:])

                # candidate row broadcast to all partitions
                candbc = cloudp.tile([P, cap], i16, tag="candbc")
                cand_src = bass.AP(
                    tensor=cands.tensor if hasattr(cands, "tensor") else cands,
                    offset=c * cap,
                    ap=[[0, P], [1, cap]],
                )
                nc.gpsimd.dma_start(out=candbc[:], in_=cand_src)

                hist = psump.tile([P, cap], f32, tag="hist")
                nc.vector.memset(hist[:], 0.0)

                def body(iv):
                    memb = ohp.tile([P, cap], bf16, tag="memb")
                    loh = ohp.tile([P, 64], bf16, tag="loh")
                    nc.vector.tensor_scalar(
                        out=memb[:], in0=candbc[:],
                        scalar1=tc14[:, bass.ds(iv, 1)], scalar2=None,
                        op0=op.is_equal,
                    )
                    nc.vector.tensor_scalar(
                        out=loh[:], in0=iota64_bf[:],
                        scalar1=tlow6[:, bass.ds(iv, 1)], scalar2=None,
                        op0=op.is_equal,
                    )
                    # transposed accumulation: hist[w, cand] += loh^T @ memb,
                    # 512-wide moving slices so the 64-wide stationary loh is
                    # shared and PE streams at full width
                    for g in range(cap // 512):
                        nc.tensor.matmul(
                            out=hist[:64, g * 512 : (g + 1) * 512],
                            lhsT=loh[:],
                            rhs=memb[:, g * 512 : (g + 1) * 512],
                            start=False, stop=True, skip_group_check=True,
                        )

                tc.For_i_unrolled(0, Fl, 1, body, max_unroll=unroll)

                # K = sum over candidates/low6 of [count >= 53]
                kps = kpsp.tile([1, cap], f32, tag="kps")
                for g in range(cap // 512):
                    mask = mkp.tile([P, 512], bf16, tag="mask")
                    nc.vector.tensor_scalar(
                        out=mask[:64, :], in0=hist[:64, g * 512 : (g + 1) * 512],
                        scalar1=52.5, scalar2=None, op0=op.is_ge,
                    )
                    nc.tensor.matmul(
                        out=kps[:1, g * 512 : (g + 1) * 512],
                        lhsT=ones_bf[:64, :], rhs=mask[:64, :],
                        start=True, stop=True,
                    )
                nc.vector.tensor_reduce(
                    out=kv_sb[:1, c : c + 1], in_=kps[:],
                    axis=mybir.AxisListType.X, op=op.add,
                )

            nc.gpsimd.dma_start(out=kvals[:, :], in_=kv_sb[:])
    nc.compile()
    return nc


def build_fused(n_clouds=CLOUDS_PER_CORE, n_points=N, nchunk=NCHUNK):
    """Single-launch kernel: coarse histogram -> on-device candidate
    compaction (threshold mask -> exclusive prefix scan via triangular
    matmuls -> position-one-hot scatter of cell ids) -> fine refine -> K.
    Outputs kvals [1, 2*n_clouds]: cols [0,n) = K, cols [n,2n) = ncand
    (host falls back if ncand > capacity)."""
    import concourse.bass as bass
    import concourse.mybir as mybir
    from concourse.tile import TileContext
    from concourse import bacc

    f32, bf16 = mybir.dt.float32, mybir.dt.bfloat16
    i16, i32 = mybir.dt.int16, mybir.dt.int32
    u16, u8 = mybir.dt.uint16, mybir.dt.uint8
    op = mybir.AluOpType
    Fl = n_points // P
    cap = nchunk * P
    BIG = 16384.0

    nc = bacc.Bacc("TRN2", target_bir_lowering=False, debug=False)
    nq = n_clouds // 4
    c14ps = [
        nc.declare_dram_parameter(f"c14p{q}", [nq, n_points], u16, isOutput=False)
        for q in range(4)
    ]
    lo6ps = [
        nc.declare_dram_parameter(f"lo6p{q}", [nq, n_points], u8, isOutput=False)
        for q in range(4)
    ]
    # constants built on the host, uploaded once and kept device-resident:
    # tri[p,j] = 1 if p<j (strictly-upper triangle), ident = I128,
    # hrow[:,j] = partition index, lrow[p,j] = j+1
    tri = nc.declare_dram_parameter("tri", [P, P], bf16, isOutput=False)
    ident = nc.declare_dram_parameter("ident", [P, P], bf16, isOutput=False)
    hrow = nc.declare_dram_parameter("hrow", [P, P], bf16, isOutput=False)
    lrow = nc.declare_dram_parameter("lrow", [P, P], bf16, isOutput=False)
    kvals = nc.declare_dram_parameter("kvals", [1, 2 * n_clouds], f32, isOutput=True)
    candscr = nc.dram_tensor("candscr", [n_clouds, cap], f32)
    ncscr = nc.dram_tensor("ncscr", [n_clouds], f32)
    kscr = nc.dram_tensor("kscr", [n_clouds, 64], f32)

    with TileContext(nc) as tc:
        with (
            tc.tile_pool(name="const", bufs=1) as constp,
            tc.tile_pool(name="raw", bufs=2) as rawp,
            tc.tile_pool(name="cloud", bufs=2) as cloudp,
            tc.tile_pool(name="cmp", bufs=1) as cmpp,
            tc.tile_pool(name="oh", bufs=8) as ohp,
            tc.tile_pool(name="mk", bufs=4) as mkp,
            tc.tile_pool(name="psA", bufs=1, space="PSUM") as psA,
            tc.tile_pool(name="psB", bufs=1, space="PSUM") as psB,
            tc.tile_pool(name="psC", bufs=1, space="PSUM") as psC,
            tc.tile_pool(name="psD", bufs=1, space="PSUM") as psD,
        ):
            iota_i = constp.tile([P, P], i32)
            nc.gpsimd.iota(iota_i[:], pattern=[[1, P]], base=0, channel_multiplier=0)
            iota_bf = constp.tile([P, P], bf16)
            nc.vector.tensor_copy(out=iota_bf[:], in_=iota_i[:])
            iota64_i = constp.tile([P, 64], i32)
            nc.gpsimd.iota(iota64_i[:], pattern=[[1, 64]], base=0, channel_multiplier=0)
            iota64_bf = constp.tile([P, 64], bf16)
            nc.vector.tensor_copy(out=iota64_bf[:], in_=iota64_i[:])
            iotacap_i32 = constp.tile([P, cap], i32)
            nc.gpsimd.iota(
                iotacap_i32[:], pattern=[[1, cap]], base=0, channel_multiplier=0
            )
            iotacap = constp.tile([P, cap], i16)
            nc.vector.tensor_copy(out=iotacap[:], in_=iotacap_i32[:])
            ones_bf = constp.tile([P, 1], bf16)
            nc.vector.memset(ones_bf[:], 1.0)
            tri_sb = constp.tile([P, P], bf16)
            nc.gpsimd.dma_start(out=tri_sb[:], in_=tri[:, :])
            ident_sb = constp.tile([P, P], bf16)
            nc.gpsimd.dma_start(out=ident_sb[:], in_=ident[:, :])
            hrow_sb = constp.tile([P, P], bf16)
            nc.gpsimd.dma_start(out=hrow_sb[:], in_=hrow[:, :])
            lrow_sb = constp.tile([P, P], bf16)
            nc.gpsimd.dma_start(out=lrow_sb[:], in_=lrow[:, :])
            kv_sb = constp.tile([1, 2 * n_clouds], f32)

            for c in range(n_clouds):
                # ---- phase 1: coarse 2^14 histogram of c14 ----
                rc = rawp.tile([P, Fl], u16, tag="rc")
                csrc = c14ps[c // nq][c % nq]
                nc.gpsimd.dma_start(out=rc[:], in_=csrc.rearrange("(p f) -> p f", p=P))
                tc14 = cloudp.tile([P, Fl], f32, tag="tc14")
                nc.vector.tensor_copy(out=tc14[:], in_=rc[:])
                thif = cloudp.tile([P, Fl], f32, tag="thif")
                nc.vector.tensor_scalar(
                    out=thif[:], in0=tc14[:], scalar1=0.0078125,
                    scalar2=0.49609375, op0=op.mult, op1=op.subtract,
                )
                thi = cloudp.tile([P, Fl], f32, tag="thi")
                nc.vector.tensor_scalar(
                    out=thi[:], in0=thif[:], scalar1=C23, scalar2=C23,
                    op0=op.add, op1=op.subtract,
                )
                tlo = cloudp.tile([P, Fl], f32, tag="tlo")
                nc.vector.scalar_tensor_tensor(
                    out=tlo[:], in0=thi[:], scalar=-128.0, in1=tc14[:],
                    op0=op.mult, op1=op.add,
                )
                h14 = psA.tile([P, P], f32, tag="ps128")
                nc.vector.memset(h14[:], 0.0)

                def body1(iv):
                    ohh = ohp.tile([P, P], bf16, tag="ohh")
                    ohl = ohp.tile([P, P], bf16, tag="ohl")
                    nc.vector.tensor_scalar(
                        out=ohh[:], in0=iota_bf[:],
                        scalar1=thi[:, bass.ds(iv, 1)], scalar2=None,
                        op0=op.is_equal,
                    )
                    nc.vector.tensor_scalar(
                        out=ohl[:], in0=iota_bf[:],
                        scalar1=tlo[:, bass.ds(iv, 1)], scalar2=None,
                        op0=op.is_equal,
                    )
                    nc.tensor.matmul(
                        out=h14[:], lhsT=ohh[:], rhs=ohl[:],
                        start=False, stop=True, skip_group_check=True,
                    )

                tc.For_i_unrolled(0, Fl, 1, body1, max_unroll=32)

                # ---- candidate compaction ----
                mask_bf = cmpp.tile([P, P], bf16, tag="mask_bf")
                nc.vector.tensor_scalar(
                    out=mask_bf[:], in0=h14[:], scalar1=52.5, scalar2=None,
                    op0=op.is_ge,
                )
                maskf = cmpp.tile([P, P], f32, tag="maskf")
                nc.vector.tensor_copy(out=maskf[:], in_=mask_bf[:])
                rowsum = cmpp.tile([P, 1], f32, tag="rowsum")
                nc.vector.tensor_reduce(
                    out=rowsum[:], in_=maskf[:], axis=mybir.AxisListType.X, op=op.add
                )
                rowsum_bf = cmpp.tile([P, 1], bf16, tag="rowsum_bf")
                nc.vector.tensor_copy(out=rowsum_bf[:], in_=rowsum[:])
                # mT = mask^T (PE transpose), then row-exclusive scan and
                # row-offset prefix via the strict upper-triangular ones
                mT_ps = psB.tile([P, P], bf16, tag="mT_ps")
                nc.tensor.transpose(mT_ps[:], mask_bf[:], ident_sb[:])
                mT_bf = cmpp.tile([P, P], bf16, tag="mT_bf")
                nc.vector.tensor_copy(out=mT_bf[:], in_=mT_ps[:])
                rowscan_ps = psA.tile([P, P], f32, tag="ps128")
                nc.tensor.matmul(
                    out=rowscan_ps[:], lhsT=mT_bf[:], rhs=tri_sb[:],
                    start=True, stop=True,
                )
                rowoff_ps = psB.tile([P, 1], f32, tag="rowoff")
                nc.tensor.matmul(
                    out=rowoff_ps[:], lhsT=tri_sb[:], rhs=rowsum_bf[:],
                    start=True, stop=True,
                )
                rowoffp = cmpp.tile([P, 1], f32, tag="rowoffp")
                nc.vector.tensor_scalar(
                    out=rowoffp[:], in0=rowoff_ps[:], scalar1=BIG, scalar2=None,
                    op0=op.add,
                )
                # ncand (inclusive prefix at the last row) -> DRAM scratch
                ncol = cmpp.tile([P, 1], f32, tag="ncol")
                nc.vector.scalar_tensor_tensor(
                    out=ncol[:], in0=rowoffp[:], scalar=1.0, in1=rowsum[:],
                    op0=op.mult, op1=op.add,
                )
                nc.gpsimd.dma_start(out=ncscr[c : c + 1], in_=ncol[P - 1, 0:1])
                # posv = rowscan + rowoff for masked cells, >= BIG otherwise
                pos1 = cmpp.tile([P, P], f32, tag="pos1")
                nc.vector.tensor_scalar(
                    out=pos1[:], in0=rowscan_ps[:], scalar1=rowoffp[:, 0:1],
                    scalar2=None, op0=op.add,
                )
                posv = cmpp.tile([P, P], f32, tag="posv")
                nc.vector.scalar_tensor_tensor(
                    out=posv[:], in0=maskf[:], scalar=-BIG, in1=pos1[:],
                    op0=op.mult, op1=op.add,
                )
                # scatter cell ids to their slots (transposed orientation:
                # stationary = one-hot chunk, moving = a [128,2] rhs staged
                # per column with the partition-index / cc+1 constants), so
                # slot s accumulates (hi, lo) at chl[s%128, 2*(s//128)+{0,1}]
                chl = psB.tile([P, 2 * nchunk], f32, tag="chl")
                nc.vector.memset(chl[:], 0.0)

                def body_sc(iv):
                    oh = ohp.tile([P, cap], bf16, tag="ohsc")
                    nc.vector.tensor_scalar(
                        out=oh[:], in0=iotacap[:],
                        scalar1=posv[:, bass.ds(iv, 1)], scalar2=None,
                        op0=op.is_equal,
                    )
                    hl2 = ohp.tile([P, 2], bf16, tag="hl2")
                    nc.vector.tensor_copy(out=hl2[:, 0:1], in_=hrow_sb[:, bass.ds(iv, 1)])
                    nc.vector.tensor_copy(out=hl2[:, 1:2], in_=lrow_sb[:, bass.ds(iv, 1)])
                    for g in range(nchunk):
                        nc.tensor.matmul(
                            out=chl[:, 2 * g : 2 * g + 2],
                            lhsT=oh[:, g * P : (g + 1) * P],
                            rhs=hl2[:], start=False, stop=True,
                            skip_group_check=True,
                        )

                tc.For_i_unrolled(0, P, 1, body_sc, max_unroll=16)

                # cand id+1 per slot = 128*hi + lo; store slot-major and
                # broadcast back to all partitions
                chl_sb = cmpp.tile([P, 2 * nchunk], f32, tag="chl_sb")
                nc.vector.tensor_copy(out=chl_sb[:], in_=chl[:])
                chl3 = chl_sb[:].rearrange("p (g t) -> p t g", t=2)
                candT = cmpp.tile([P, nchunk], f32, tag="candT")
                nc.vector.scalar_tensor_tensor(
                    out=candT[:], in0=chl3[:, 0], scalar=128.0, in1=chl3[:, 1],
                    op0=op.mult, op1=op.add,
                )
                nc.gpsimd.dma_start(
                    out=candscr[c].rearrange("(g p) -> p g", p=P), in_=candT[:]
                )
                cand_f = cmpp.tile([P, cap], f32, tag="cand_f")
                cst = candscr.tensor if hasattr(candscr, "tensor") else candscr
                nc.gpsimd.dma_start(
                    out=cand_f[:],
                    in_=bass.AP(tensor=cst, offset=c * cap, ap=[[0, P], [1, cap]]),
                )
                candbc = cmpp.tile([P, cap], i16, tag="candbc")
                nc.vector.tensor_copy(out=candbc[:], in_=cand_f[:])

                # ---- phase 2: fine counts on candidate cells ----
                tc14p1 = cloudp.tile([P, Fl], f32, tag="tc14p1")
                nc.vector.tensor_scalar(
                    out=tc14p1[:], in0=rc[:], scalar1=1.0, scalar2=None, op0=op.add
                )
                rl = rawp.tile([P, Fl], u8, tag="rl")
                lsrc = lo6ps[c // nq][c % nq]
                nc.gpsimd.dma_start(out=rl[:], in_=lsrc.rearrange("(p f) -> p f", p=P))
                tlow6 = cloudp.tile([P, Fl], f32, tag="tlow6")
                nc.vector.tensor_copy(out=tlow6[:], in_=rl[:])

                hist = psD.tile([P, cap], f32, tag="hist")
                nc.vector.memset(hist[:], 0.0)

                def body2(iv):
                    memb = ohp.tile([P, cap], bf16, tag="memb")
                    loh = ohp.tile([P, 64], bf16, tag="loh")
                    nc.vector.tensor_scalar(
                        out=memb[:], in0=candbc[:],
                        scalar1=tc14p1[:, bass.ds(iv, 1)], scalar2=None,
                        op0=op.is_equal,
                    )
                    nc.vector.tensor_scalar(
                        out=loh[:], in0=iota64_bf[:],
                        scalar1=tlow6[:, bass.ds(iv, 1)], scalar2=None,
                        op0=op.is_equal,
                    )
                    for g in range(cap // 512):
                        sl = slice(g * 512, (g + 1) * 512)
                        nc.tensor.matmul(
                            out=hist[:64, sl], lhsT=loh[:], rhs=memb[:, sl],
                            start=False, stop=True, skip_group_check=True,
                        )

                tc.For_i_unrolled(0, Fl, 1, body2, max_unroll=16)

                # K = #(count >= 53): per-partition counts, then a DRAM
                # roundtrip to land the 64 partials on partition 0
                maskk = mkp.tile([P, cap], bf16, tag="maskk")
                nc.vector.tensor_scalar(
                    out=maskk[:64, :], in0=hist[:64, :],
                    scalar1=52.5, scalar2=None, op0=op.is_ge,
                )
                ks = mkp.tile([P, 1], f32, tag="ks")
                nc.vector.tensor_reduce(
                    out=ks[:64, :], in_=maskk[:64, :],
                    axis=mybir.AxisListType.X, op=op.add,
                )
                nc.gpsimd.dma_start(
                    out=kscr[c].rearrange("(p o) -> p o", o=1), in_=ks[:64, 0:1]
                )

            for c in range(n_clouds):
                kr = constp.tile([1, 64], f32, tag=f"kr{c}")
                nc.gpsimd.dma_start(
                    out=kr[:], in_=kscr[c].rearrange("(o f) -> o f", o=1)
                )
                nc.vector.tensor_reduce(
                    out=kv_sb[:1, c : c + 1], in_=kr[:],
                    axis=mybir.AxisListType.X, op=op.add,
                )

            # ncand values (still offset by BIG) -> row 0, cols [n, 2n)
            ncrow = constp.tile([1, n_clouds], f32)
            nc.gpsimd.dma_start(
                out=ncrow[:], in_=ncscr[:].rearrange("(o b) -> o b", o=1)
            )
            nc.vector.tensor_scalar(
                out=kv_sb[:1, n_clouds : 2 * n_clouds], in0=ncrow[:],
                scalar1=BIG, scalar2=None, op0=op.subtract,
            )
            nc.gpsimd.dma_start(out=kvals[:, :], in_=kv_sb[:])
    nc.compile()
    return nc


class _Runner:
    """jit(shard_map(bass_exec)) callable over 8 cores with device-resident
    inputs.  Mirrors concourse.bass2jax.run_bass_via_pjrt's lowering (the
    @via_axon target of bass_utils.run_bass_kernel_spmd), but accepts jax
    Arrays already placed on the devices so repeated launches don't re-ship
    inputs, and keeps the (never-donated, fully-overwritten) output
    parameter slots device-resident too."""

    def __init__(self, nc, n_cores=NCORES):
        import jax
        from concourse import bass2jax
        import concourse.mybir as mybir
        from jax.experimental.shard_map import shard_map
        from jax.sharding import Mesh, PartitionSpec, NamedSharding

        bass2jax.install_neuronx_cc_hook()
        assert not nc.dbg_callbacks if nc.dbg_addr is not None else True
        partition_name = (
            nc.partition_id_tensor.name if nc.partition_id_tensor else None
        )
        self.jax = jax
        self.n_cores = n_cores
        devices = jax.devices()[:n_cores]
        assert len(devices) == n_cores
        self.devices = devices
        self.mesh = Mesh(np.asarray(devices), ("core",))
        self.sharding = NamedSharding(self.mesh, PartitionSpec("core"))

        in_names, out_names, out_avals = [], [], []
        in_meta = {}
        for alloc in nc.m.functions[0].allocations:
            if not isinstance(alloc, mybir.MemoryLocationSet):
                continue
            name = alloc.memorylocations[0].name
            if alloc.kind == "ExternalInput":
                if name == partition_name:
                    continue
                in_names.append(name)
                in_meta[name] = (tuple(alloc.tensor_shape), mybir.dt.np(alloc.dtype))
            elif alloc.kind == "ExternalOutput":
                out_names.append(name)
                out_avals.append(
                    jax.core.ShapedArray(
                        tuple(alloc.tensor_shape), mybir.dt.np(alloc.dtype)
                    )
                )
        self.in_names, self.out_names = in_names, out_names
        self.in_meta = in_meta
        all_in = tuple(in_names) + tuple(out_names)
        if partition_name is not None:
            all_in = all_in + (partition_name,)

        def _body(*args):
            operands = list(args)
            if partition_name is not None:
                operands.append(bass2jax.partition_id_tensor())
            outs = bass2jax._bass_exec_p.bind(
                *operands,
                out_avals=tuple(out_avals),
                in_names=all_in,
                out_names=tuple(out_names),
                lowering_input_output_aliases=(),
                sim_require_finite=True,
                sim_require_nnan=True,
                nc=nc,
            )
            return tuple(outs)

        pspec = PartitionSpec("core")
        n_args = len(in_names) + len(out_names)
        self.fn = jax.jit(
            shard_map(
                _body,
                mesh=self.mesh,
                in_specs=(pspec,) * n_args,
                out_specs=(pspec,) * len(out_names),
                check_rep=False,
            ),
            keep_unused=True,
        )
        # persistent device-resident buffers for the output parameter slots
        # (never donated; the kernels fully overwrite every output element)
        self.out_dummies = [
            jax.device_put(
                np.zeros((n_cores * av.shape[0], *av.shape[1:]), av.dtype),
                self.sharding,
            )
            for av in out_avals
        ]
        self.extra = {}

    def __call__(self, arrays):
        args = []
        for name in self.in_names:
            if name in arrays:
                args.append(arrays[name])
            else:
                if name not in self.extra:
                    shape, dt = self.in_meta[name]
                    z = np.zeros((self.n_cores * shape[0], *shape[1:]), dt)
                    self.extra[name] = self.jax.device_put(z, self.sharding)
                args.append(self.extra[name])
        outs = self.fn(*args, *self.out_dummies)
        return dict(zip(self.out_names, outs))


def _state():
    if "st" in _cache:
        return _cache["st"]
    import jax
    import ml_dtypes

    ncf = build_fused()
    runf = _Runner(ncf)
    # host-built constants, uploaded once and kept device-resident (each
    # core gets its own copy: global shape [8*128, 128])
    bf = ml_dtypes.bfloat16
    pidx = np.arange(P)
    tri = (pidx[:, None] < pidx[None, :]).astype(bf)
    ident = np.eye(P).astype(bf)
    hrow = np.broadcast_to(pidx[:, None], (P, P)).astype(bf)
    lrow = np.broadcast_to(pidx[None, :] + 1, (P, P)).astype(bf)
    consts = {
        name: jax.device_put(np.ascontiguousarray(np.tile(a, (NCORES, 1))), runf.sharding)
        for name, a in (("tri", tri), ("ident", ident), ("hrow", hrow), ("lrow", lrow))
    }
    BQ = B // 4
    st = {
        "jax": jax,
        "runf": runf,
        "consts": consts,
        "devices": runf.devices,
        "sharding": runf.sharding,
        # persistent host work buffers (avoid first-touch page faults on the
        # timed warm call)
        "t": np.empty((BQ, N, 2), np.float32),
        "sf": np.empty((BQ, N), np.float32),
        "zi": np.empty((BQ, N), np.int16),
        "c14": np.empty((B, N), np.uint16),
        "lo6": np.empty((B, N), np.uint8),
        # c14 = floor(s/64) = 16*qx + floor(qz/64): exact in f32 (value
        # < 2^14, qz/64 on a 2^-6 grid, f32 ulp at 2^14 is 2^-9), and the
        # truncating u16 cast is the floor
        "wc14": np.asarray([16.0, 0.015625], np.float32),
        # device slot (core i, local cloud c) <-> original batch index:
        # quarter q = c//2 holds original clouds [16q, 16q+16), two per core
        "order": np.asarray(
            [
                16 * (c // 2) + 2 * i + (c % 2)
                for i in range(NCORES)
                for c in range(CLOUDS_PER_CORE)
            ]
        ),
    }
    _cache["st"] = st
    return st


def _quant_quarter(st, pcd_q, qi):
    """Quantize one batch-quarter [B/4, N, 2] into the c14/lo6 plane slices.
    Exact: q = rint(1000*p) in f32 (matches jnp.round); with s = qx*1024+qz,
    c14 = s>>6 = 16*qx + (qz>>6) and lo6 = s&63 = qz&63 (1024 = 0 mod 64)."""
    BQ = B // 4
    sl = slice(qi * BQ, (qi + 1) * BQ)
    t = st["t"]
    np.multiply(pcd_q, np.float32(1000.0), out=t)
    np.rint(t, out=t)
    mn = t.min(axis=1)
    mx = t.max(axis=1)
    good = (mn[:, 0] >= 0) & (mn[:, 1] >= 0) & (mx[:, 0] <= 1023) & (mx[:, 1] <= 1023)
    if not good.all():
        t[~good] = 0.0  # keep device indices in range; host recomputes these
    sf, zi = st["sf"], st["zi"]
    np.dot(t.reshape(-1, 2), st["wc14"], out=sf.reshape(-1))
    np.copyto(st["c14"][sl], sf, casting="unsafe")
    np.copyto(zi, t[..., 1], casting="unsafe")
    np.bitwise_and(zi, 63, out=zi)
    np.copyto(st["lo6"][sl], zi, casting="unsafe")
    return good


def _host_exact(points):
    """Exact numpy replica of the reference for one cloud. [N,2] f32 -> [TOPK]."""
    q = np.round(np.float32(1000.0) * points.astype(np.float32))
    xi = (q[:, 0] - q[:, 0].min()).astype(np.int64)
    zi = (q[:, 1] - q[:, 1].min()).astype(np.int64)
    idx = xi * GZ + zi
    counts = np.bincount(idx, minlength=1024 * GZ).astype(np.float32)
    occ = counts / np.float32(points.shape[0]) > np.float32(0.0002)
    k = min(int(occ.sum()), TOPK)
    out = np.zeros((TOPK,), np.float32)
    out[:k] = 1.0
    return out


def kernel(pcd):
    import time

    t_start = time.time()
    pcd = np.ascontiguousarray(np.asarray(pcd), dtype=np.float32)
    assert pcd.shape == (B, N, 2), pcd.shape
    st = _state()
    jax = st["jax"]
    sharding = st["sharding"]
    _dbg("state ready", t_start)

    # pipeline: quantize batch-quarter q on the (single) CPU while a single
    # uploader thread streams finished quarters over the tunnel (sharded
    # device_put blocks until the transfer lands, so it must live off the
    # main thread; one uploader keeps transfers big and in order).  c14 is
    # queued before lo6 each round: phase 1 only reads c14, so it can run
    # while the lo6 tail is still in flight.
    from concurrent.futures import ThreadPoolExecutor

    BQ = B // 4
    goods = [None] * 4
    quarters = pcd.reshape(4, BQ, N, 2)
    with ThreadPoolExecutor(max_workers=1) as ex:
        futs_c = [None] * 4
        futs_l = [None] * 4
        for qi in range(4):
            goods[qi] = _quant_quarter(st, quarters[qi], qi)
            sl = slice(qi * BQ, (qi + 1) * BQ)
            futs_c[qi] = ex.submit(jax.device_put, st["c14"][sl], sharding)
            futs_l[qi] = ex.submit(jax.device_put, st["lo6"][sl], sharding)
        good = np.concatenate(goods)
        _dbg("quantize done, uploads in flight", t_start)
        c14_d = [f.result() for f in futs_c]
        lo6_d = [f.result() for f in futs_l]
    _dbg("uploads done", t_start)

    args = {f"c14p{q}": c14_d[q] for q in range(4)}
    args.update({f"lo6p{q}": lo6_d[q] for q in range(4)})
    args.update(st["consts"])
    r = st["runf"](args)
    kv = np.asarray(r["kvals"]).reshape(NCORES, 2 * CLOUDS_PER_CORE)
    _dbg("fused done", t_start)
    kslot = kv[:, :CLOUDS_PER_CORE].reshape(B)  # device-slot order
    ncand = kv[:, CLOUDS_PER_CORE:].reshape(B)
    order = st["order"]  # slot g -> original batch index
    good_slot = good[order]
    ok = good_slot & (ncand <= CAND_CAP)

    out = np.zeros((B, TOPK, 1), np.float32)
    iota = np.arange(TOPK)
    for g in range(B):
        b = int(order[g])
        if ok[g]:
            out[b, :, 0] = iota < kslot[g]
        else:
            out[b, :, 0] = _host_exact(pcd[b])
    _dbg("assembled", t_start)
    return out


# revision 38
# speedup vs baseline: 1.5864x; 1.3521x over previous
"""Trainium2 Bass kernel for nn_DeepMapping2D (histogram_binning).

Reference semantics: per cloud, quantize points to integer mm bins
(q = round_half_even(1000*p)), histogram into a 1024x1024 grid (shifted by
per-cloud coordinate minima), threshold counts (count/N > 2e-4 <=> count>=53),
sort the 0/1 occupancy descending, truncate to TOPK.  The sorted vector is K
ones then zeros, K = #bins with count >= 53.  Shifting by the minima is a
bijection on occupied bins, so K is shift-invariant and the device can work
on unshifted bin ids s = qx*1024 + qz (fine id, < 2^20).

Device algorithm (exact, two launches, all heavy counting on device):

Phase 1 (screen): per cloud, the exact 2^14-bin coarse histogram H14 over
c14 = s>>6, computed as a PSUM-matmul scatter: per column of 128 points,
build 128-wide one-hots of hi7=c14>>7 and lo7=c14&127 by comparing a
constant iota row against the point's value (DVE tensor_scalar is_equal with
a per-partition scalar), then accumulate onehot_hi^T @ onehot_lo into PSUM
(bf16 0/1 inputs are exact; fp32 accumulation).  H14, clamped to u8, goes
back to the host (1 MB total).

Host: candidate cells = {c14 : H14[c14] >= 53} (every fine bin with count
>= 53 lives in one, since H14 upper-bounds its 64 fine bins).  ~1.4k/cloud
for the rbg-generated inputs.  Padded with -1 to NCHUNK*128 int16.

Phase 2 (refine): per cloud, exact fine counts for every candidate cell:
per column, one membership one-hot against the candidate row (int16
candidates streamed at DVE 4x, compared against the point's c14 as the
per-partition scalar) and one 64-wide one-hot of low6 = s&63; NCHUNK
matmuls accumulate membership^T @ onehot_low6 into PSUM -> exact
[candidate, low6] fine counts.  Threshold >= 53, count via ones^T @ mask
matmul, giving K per cloud.  The host formats the final rows (K ones then
zeros) from the device-computed K values.

Transport optimization (the axon tunnel runs at ~35 MB/s, so bytes moved
dominate wall time): the host quantizes once into two integer planes,
c14 = s>>6 (uint16) and lo6 = s&63 (uint8) - 3 B/point = 50 MB instead of
the 8 B/point raw floats - and uploads them a single time.  Both phases
run through a jit(shard_map(bass_exec)) callable (the same primitive
bass_utils.run_bass_kernel_spmd lowers to under axon) against the SAME
device-resident plane arrays, so phase 2 re-reads them from device DRAM
instead of re-shipping 128 MB.  Quantization is pipelined per core-shard
with the uploads.

Host guards keep the kernel exact for arbitrary inputs: clouds with
coordinates outside [0, 1023] mm (or counts exceeding the candidate
capacity) fall back to an exact numpy recomputation of that cloud.

Sharding: data-parallel over batch: 64 clouds -> 8 cores x 8 clouds.
"""

import os
import numpy as np

B = 64
N = 262144
TOPK = 5120
NCORES = 8
CLOUDS_PER_CORE = B // NCORES
P = 128
F = N // P
GZ = 1024
NCHUNK = 12  # candidate capacity = NCHUNK*128 cells per cloud
CAND_CAP = NCHUNK * P
THRESH_COUNT = 53.0
C23 = 12582912.0  # 1.5 * 2^23

_cache = {}
_DEBUG = os.environ.get("KERNEL_DEBUG", "0") == "1"


def _dbg(msg, t0=None):
    if _DEBUG:
        import time

        if t0 is not None:
            print(f"[kernel] {msg}: {time.time()-t0:.3f}s", flush=True)
        else:
            print(f"[kernel] {msg}", flush=True)


def build_phase1(n_clouds=CLOUDS_PER_CORE, n_points=N, unroll=32):
    """Per-cloud exact 2^14-bin coarse histogram -> DRAM (u8, clamped)."""
    import concourse.bass as bass
    import concourse.mybir as mybir
    from concourse.tile import TileContext
    from concourse import bacc

    f32, bf16 = mybir.dt.float32, mybir.dt.bfloat16
    i32, u16, u8 = mybir.dt.int32, mybir.dt.uint16, mybir.dt.uint8
    op = mybir.AluOpType
    Fl = n_points // P

    nc = bacc.Bacc("TRN2", target_bir_lowering=False, debug=False)
    nq = n_clouds // 4
    c14ps = [
        nc.declare_dram_parameter(f"c14p{q}", [nq, n_points], u16, isOutput=False)
        for q in range(4)
    ]
    h14 = nc.declare_dram_parameter("h14", [n_clouds, P, P], u8, isOutput=True)

    with TileContext(nc) as tc:
        with (
            tc.tile_pool(name="const", bufs=1) as constp,
            tc.tile_pool(name="raw", bufs=2) as rawp,
            tc.tile_pool(name="chain", bufs=2) as chainp,
            tc.tile_pool(name="hilo", bufs=1) as hilop,
            tc.tile_pool(name="oh", bufs=8) as ohp,
            tc.tile_pool(name="hout", bufs=2) as houtp,
            tc.tile_pool(name="psum", bufs=1, space="PSUM") as psump,
        ):
            iota_i = constp.tile([P, P], i32)
            nc.gpsimd.iota(iota_i[:], pattern=[[1, P]], base=0, channel_multiplier=0)
            iota_bf = constp.tile([P, P], bf16)
            nc.vector.tensor_copy(out=iota_bf[:], in_=iota_i[:])

            this, tlos, hists = [], [], []
            for c in range(n_clouds):
                rc = rawp.tile([P, Fl], u16, tag="rc")
                src = c14ps[c // nq][c % nq]
                nc.gpsimd.dma_start(out=rc[:], in_=src.rearrange("(p f) -> p f", p=P))
                tc14 = chainp.tile([P, Fl], f32, tag="tc14")
                nc.vector.tensor_copy(out=tc14[:], in_=rc[:])
                # hi7 = floor(c14/128); lo7 = c14 - 128*hi7
                thif = chainp.tile([P, Fl], f32, tag="thif")
                nc.vector.tensor_scalar(
                    out=thif[:], in0=tc14[:], scalar1=0.0078125,
                    scalar2=0.49609375, op0=op.mult, op1=op.subtract,
                )
                thi = hilop.tile([P, Fl], f32, tag=f"thi{c}")
                nc.vector.tensor_scalar(
                    out=thi[:], in0=thif[:], scalar1=C23, scalar2=C23,
                    op0=op.add, op1=op.subtract,
                )
                tlo = hilop.tile([P, Fl], f32, tag=f"tlo{c}")
                nc.vector.scalar_tensor_tensor(
                    out=tlo[:], in0=thi[:], scalar=-128.0, in1=tc14[:],
                    op0=op.mult, op1=op.add,
                )
                this.append(thi)
                tlos.append(tlo)
                hist = psump.tile([P, P], f32, tag=f"hist{c}")
                nc.vector.memset(hist[:], 0.0)
                hists.append(hist)

            def body(iv):
                for c in range(n_clouds):
                    ohh = ohp.tile([P, P], bf16, tag="ohh")
                    ohl = ohp.tile([P, P], bf16, tag="ohl")
                    nc.vector.tensor_scalar(
                        out=ohh[:], in0=iota_bf[:],
                        scalar1=this[c][:, bass.ds(iv, 1)], scalar2=None,
                        op0=op.is_equal,
                    )
                    nc.vector.tensor_scalar(
                        out=ohl[:], in0=iota_bf[:],
                        scalar1=tlos[c][:, bass.ds(iv, 1)], scalar2=None,
                        op0=op.is_equal,
                    )
                    nc.tensor.matmul(
                        out=hists[c][:], lhsT=ohh[:], rhs=ohl[:],
                        start=False, stop=True, skip_group_check=True,
                    )

            tc.For_i_unrolled(0, Fl, 1, body, max_unroll=unroll)

            for c in range(n_clouds):
                hcl = houtp.tile([P, P], f32, tag="hcl")
                nc.vector.tensor_scalar(
                    out=hcl[:], in0=hists[c][:], scalar1=255.0, scalar2=None,
                    op0=op.min,
                )
                hu8 = houtp.tile([P, P], u8, tag="hu8")
                nc.vector.tensor_copy(out=hu8[:], in_=hcl[:])
                nc.gpsimd.dma_start(out=h14[c], in_=hu8[:])
    nc.compile()
    return nc


def build_phase2(n_clouds=CLOUDS_PER_CORE, n_points=N, nchunk=NCHUNK, unroll=16):
    """Exact [candidate,64] fine counts -> K per cloud."""
    import concourse.bass as bass
    import concourse.mybir as mybir
    from concourse.tile import TileContext
    from concourse import bacc

    f32, bf16 = mybir.dt.float32, mybir.dt.bfloat16
    i16, i32 = mybir.dt.int16, mybir.dt.int32
    u16, u8 = mybir.dt.uint16, mybir.dt.uint8
    op = mybir.AluOpType
    Fl = n_points // P
    cap = nchunk * P

    nc = bacc.Bacc("TRN2", target_bir_lowering=False, debug=False)
    nq = n_clouds // 4
    c14ps = [
        nc.declare_dram_parameter(f"c14p{q}", [nq, n_points], u16, isOutput=False)
        for q in range(4)
    ]
    lo6ps = [
        nc.declare_dram_parameter(f"lo6p{q}", [nq, n_points], u8, isOutput=False)
        for q in range(4)
    ]
    cands = nc.declare_dram_parameter("cands", [n_clouds, cap], i16, isOutput=False)
    kvals = nc.declare_dram_parameter("kvals", [1, n_clouds], f32, isOutput=True)

    with TileContext(nc) as tc:
        with (
            tc.tile_pool(name="const", bufs=1) as constp,
            tc.tile_pool(name="raw", bufs=2) as rawp,
            tc.tile_pool(name="cloud", bufs=2) as cloudp,
            tc.tile_pool(name="oh", bufs=8) as ohp,
            tc.tile_pool(name="mk", bufs=4) as mkp,
            tc.tile_pool(name="psum", bufs=1, space="PSUM") as psump,
            tc.tile_pool(name="kps", bufs=1, space="PSUM") as kpsp,
        ):
            iota64_i = constp.tile([P, 64], i32)
            nc.gpsimd.iota(iota64_i[:], pattern=[[1, 64]], base=0, channel_multiplier=0)
            iota64_bf = constp.tile([P, 64], bf16)
            nc.vector.tensor_copy(out=iota64_bf[:], in_=iota64_i[:])
            ones_bf = constp.tile([P, 1], bf16)
            nc.vector.memset(ones_bf[:], 1.0)
            kv_sb = constp.tile([1, n_clouds], f32)

            for c in range(n_clouds):
                rc = rawp.tile([P, Fl], u16, tag="rc")
                csrc = c14ps[c // nq][c % nq]
                nc.gpsimd.dma_start(out=rc[:], in_=csrc.rearrange("(p f) -> p f", p=P))
                tc14 = cloudp.tile([P, Fl], f32, tag="tc14")
                nc.vector.tensor_copy(out=tc14[:], in_=rc[:])
                rl = rawp.tile([P, Fl], u8, tag="rl")
                lsrc = lo6ps[c // nq][c % nq]
                nc.gpsimd.dma_start(out=rl[:], in_=lsrc.rearrange("(p f) -> p f", p=P))
                tlow6 = cloudp.tile([P, Fl], f32, tag="tlow6")
                nc.vector.tensor_copy(out=tlow6[:], in_=rl[:])

                # candidate row broadcast to all partitions
                candbc = cloudp.tile([P, cap], i16, tag="candbc")
                cand_src = bass.AP(
                    tensor=cands.tensor if hasattr(cands, "tensor") else cands,
                    offset=c * cap,
                    ap=[[0, P], [1, cap]],
                )
                nc.gpsimd.dma_start(out=candbc[:], in_=cand_src)

                hist = psump.tile([P, cap], f32, tag="hist")
                nc.vector.memset(hist[:], 0.0)

                def body(iv):
                    memb = ohp.tile([P, cap], bf16, tag="memb")
                    loh = ohp.tile([P, 64], bf16, tag="loh")
                    nc.vector.tensor_scalar(
                        out=memb[:], in0=candbc[:],
                        scalar1=tc14[:, bass.ds(iv, 1)], scalar2=None,
                        op0=op.is_equal,
                    )
                    nc.vector.tensor_scalar(
                        out=loh[:], in0=iota64_bf[:],
                        scalar1=tlow6[:, bass.ds(iv, 1)], scalar2=None,
                        op0=op.is_equal,
                    )
                    # transposed accumulation: hist[w, cand] += loh^T @ memb,
                    # 512-wide moving slices so the 64-wide stationary loh is
                    # shared and PE streams at full width
                    for g in range(cap // 512):
                        nc.tensor.matmul(
                            out=hist[:64, g * 512 : (g + 1) * 512],
                            lhsT=loh[:],
                            rhs=memb[:, g * 512 : (g + 1) * 512],
                            start=False, stop=True, skip_group_check=True,
                        )

                tc.For_i_unrolled(0, Fl, 1, body, max_unroll=unroll)

                # K = sum over candidates/low6 of [count >= 53]
                kps = kpsp.tile([1, cap], f32, tag="kps")
                for g in range(cap // 512):
                    mask = mkp.tile([P, 512], bf16, tag="mask")
                    nc.vector.tensor_scalar(
                        out=mask[:64, :], in0=hist[:64, g * 512 : (g + 1) * 512],
                        scalar1=52.5, scalar2=None, op0=op.is_ge,
                    )
                    nc.tensor.matmul(
                        out=kps[:1, g * 512 : (g + 1) * 512],
                        lhsT=ones_bf[:64, :], rhs=mask[:64, :],
                        start=True, stop=True,
                    )
                nc.vector.tensor_reduce(
                    out=kv_sb[:1, c : c + 1], in_=kps[:],
                    axis=mybir.AxisListType.X, op=op.add,
                )

            nc.gpsimd.dma_start(out=kvals[:, :], in_=kv_sb[:])
    nc.compile()
    return nc


def build_fused(n_clouds=CLOUDS_PER_CORE, n_points=N, nchunk=NCHUNK, nparams=4):
    """Single-launch kernel: coarse histogram -> on-device candidate
    compaction (threshold mask -> exclusive prefix scan via triangular
    matmuls -> position-one-hot scatter of cell ids) -> fine refine -> K.
    Outputs kvals [1, 2*n_clouds]: cols [0,n) = K, cols [n,2n) = ncand
    (host falls back if ncand > capacity)."""
    import concourse.bass as bass
    import concourse.mybir as mybir
    from concourse.tile import TileContext
    from concourse import bacc

    f32, bf16 = mybir.dt.float32, mybir.dt.bfloat16
    i16, i32 = mybir.dt.int16, mybir.dt.int32
    u16, u8 = mybir.dt.uint16, mybir.dt.uint8
    op = mybir.AluOpType
    Fl = n_points // P
    cap = nchunk * P
    BIG = 16384.0

    nc = bacc.Bacc("TRN2", target_bir_lowering=False, debug=False)
    nq = n_clouds // nparams
    c14ps = [
        nc.declare_dram_parameter(f"c14p{q}", [nq, n_points], u16, isOutput=False)
        for q in range(nparams)
    ]
    lo6ps = [
        nc.declare_dram_parameter(f"lo6p{q}", [nq, n_points], u8, isOutput=False)
        for q in range(nparams)
    ]
    # constants built on the host, uploaded once and kept device-resident:
    # tri[p,j] = 1 if p<j (strictly-upper triangle), ident = I128,
    # hrow[:,j] = partition index, lrow[p,j] = j+1
    tri = nc.declare_dram_parameter("tri", [P, P], bf16, isOutput=False)
    ident = nc.declare_dram_parameter("ident", [P, P], bf16, isOutput=False)
    hrow = nc.declare_dram_parameter("hrow", [P, P], bf16, isOutput=False)
    lrow = nc.declare_dram_parameter("lrow", [P, P], bf16, isOutput=False)
    kvals = nc.declare_dram_parameter("kvals", [1, 2 * n_clouds], f32, isOutput=True)
    candscr = nc.dram_tensor("candscr", [n_clouds, cap], f32)
    ncscr = nc.dram_tensor("ncscr", [n_clouds], f32)
    kscr = nc.dram_tensor("kscr", [n_clouds, 64], f32)

    with TileContext(nc) as tc:
        with (
            tc.tile_pool(name="const", bufs=1) as constp,
            tc.tile_pool(name="raw", bufs=2) as rawp,
            tc.tile_pool(name="cloud", bufs=2) as cloudp,
            tc.tile_pool(name="cmp", bufs=1) as cmpp,
            tc.tile_pool(name="oh", bufs=8) as ohp,
            tc.tile_pool(name="mk", bufs=4) as mkp,
            tc.tile_pool(name="psA", bufs=1, space="PSUM") as psA,
            tc.tile_pool(name="psB", bufs=1, space="PSUM") as psB,
            tc.tile_pool(name="psC", bufs=1, space="PSUM") as psC,
            tc.tile_pool(name="psD", bufs=1, space="PSUM") as psD,
        ):
            iota_i = constp.tile([P, P], i32)
            nc.gpsimd.iota(iota_i[:], pattern=[[1, P]], base=0, channel_multiplier=0)
            iota_bf = constp.tile([P, P], bf16)
            nc.vector.tensor_copy(out=iota_bf[:], in_=iota_i[:])
            iota64_i = constp.tile([P, 64], i32)
            nc.gpsimd.iota(iota64_i[:], pattern=[[1, 64]], base=0, channel_multiplier=0)
            iota64_bf = constp.tile([P, 64], bf16)
            nc.vector.tensor_copy(out=iota64_bf[:], in_=iota64_i[:])
            iotacap_i32 = constp.tile([P, cap], i32)
            nc.gpsimd.iota(
                iotacap_i32[:], pattern=[[1, cap]], base=0, channel_multiplier=0
            )
            iotacap = constp.tile([P, cap], i16)
            nc.vector.tensor_copy(out=iotacap[:], in_=iotacap_i32[:])
            ones_bf = constp.tile([P, 1], bf16)
            nc.vector.memset(ones_bf[:], 1.0)
            tri_sb = constp.tile([P, P], bf16)
            nc.gpsimd.dma_start(out=tri_sb[:], in_=tri[:, :])
            ident_sb = constp.tile([P, P], bf16)
            nc.gpsimd.dma_start(out=ident_sb[:], in_=ident[:, :])
            hrow_sb = constp.tile([P, P], bf16)
            nc.gpsimd.dma_start(out=hrow_sb[:], in_=hrow[:, :])
            lrow_sb = constp.tile([P, P], bf16)
            nc.gpsimd.dma_start(out=lrow_sb[:], in_=lrow[:, :])
            kv_sb = constp.tile([1, 2 * n_clouds], f32)

            for c in range(n_clouds):
                # ---- phase 1: coarse 2^14 histogram of c14 ----
                rc = rawp.tile([P, Fl], u16, tag="rc")
                csrc = c14ps[c // nq][c % nq]
                nc.gpsimd.dma_start(out=rc[:], in_=csrc.rearrange("(p f) -> p f", p=P))
                tc14 = cloudp.tile([P, Fl], f32, tag="tc14")
                nc.vector.tensor_copy(out=tc14[:], in_=rc[:])
                thif = cloudp.tile([P, Fl], f32, tag="thif")
                nc.vector.tensor_scalar(
                    out=thif[:], in0=tc14[:], scalar1=0.0078125,
                    scalar2=0.49609375, op0=op.mult, op1=op.subtract,
                )
                thi = cloudp.tile([P, Fl], f32, tag="thi")
                nc.vector.tensor_scalar(
                    out=thi[:], in0=thif[:], scalar1=C23, scalar2=C23,
                    op0=op.add, op1=op.subtract,
                )
                tlo = cloudp.tile([P, Fl], f32, tag="tlo")
                nc.vector.scalar_tensor_tensor(
                    out=tlo[:], in0=thi[:], scalar=-128.0, in1=tc14[:],
                    op0=op.mult, op1=op.add,
                )
                h14 = psA.tile([P, P], f32, tag="ps128")
                nc.vector.memset(h14[:], 0.0)

                def body1(iv):
                    ohh = ohp.tile([P, P], bf16, tag="ohh")
                    ohl = ohp.tile([P, P], bf16, tag="ohl")
                    nc.vector.tensor_scalar(
                        out=ohh[:], in0=iota_bf[:],
                        scalar1=thi[:, bass.ds(iv, 1)], scalar2=None,
                        op0=op.is_equal,
                    )
                    nc.vector.tensor_scalar(
                        out=ohl[:], in0=iota_bf[:],
                        scalar1=tlo[:, bass.ds(iv, 1)], scalar2=None,
                        op0=op.is_equal,
                    )
                    nc.tensor.matmul(
                        out=h14[:], lhsT=ohh[:], rhs=ohl[:],
                        start=False, stop=True, skip_group_check=True,
                    )

                tc.For_i_unrolled(0, Fl, 1, body1, max_unroll=32)

                # ---- candidate compaction ----
                mask_bf = cmpp.tile([P, P], bf16, tag="mask_bf")
                nc.vector.tensor_scalar(
                    out=mask_bf[:], in0=h14[:], scalar1=52.5, scalar2=None,
                    op0=op.is_ge,
                )
                maskf = cmpp.tile([P, P], f32, tag="maskf")
                nc.vector.tensor_copy(out=maskf[:], in_=mask_bf[:])
                rowsum = cmpp.tile([P, 1], f32, tag="rowsum")
                nc.vector.tensor_reduce(
                    out=rowsum[:], in_=maskf[:], axis=mybir.AxisListType.X, op=op.add
                )
                rowsum_bf = cmpp.tile([P, 1], bf16, tag="rowsum_bf")
                nc.vector.tensor_copy(out=rowsum_bf[:], in_=rowsum[:])
                # mT = mask^T (PE transpose), then row-exclusive scan and
                # row-offset prefix via the strict upper-triangular ones
                mT_ps = psB.tile([P, P], bf16, tag="mT_ps")
                nc.tensor.transpose(mT_ps[:], mask_bf[:], ident_sb[:])
                mT_bf = cmpp.tile([P, P], bf16, tag="mT_bf")
                nc.vector.tensor_copy(out=mT_bf[:], in_=mT_ps[:])
                rowscan_ps = psA.tile([P, P], f32, tag="ps128")
                nc.tensor.matmul(
                    out=rowscan_ps[:], lhsT=mT_bf[:], rhs=tri_sb[:],
                    start=True, stop=True,
                )
                rowoff_ps = psB.tile([P, 1], f32, tag="rowoff")
                nc.tensor.matmul(
                    out=rowoff_ps[:], lhsT=tri_sb[:], rhs=rowsum_bf[:],
                    start=True, stop=True,
                )
                rowoffp = cmpp.tile([P, 1], f32, tag="rowoffp")
                nc.vector.tensor_scalar(
                    out=rowoffp[:], in0=rowoff_ps[:], scalar1=BIG, scalar2=None,
                    op0=op.add,
                )
                # ncand (inclusive prefix at the last row) -> DRAM scratch
                ncol = cmpp.tile([P, 1], f32, tag="ncol")
                nc.vector.scalar_tensor_tensor(
                    out=ncol[:], in0=rowoffp[:], scalar=1.0, in1=rowsum[:],
                    op0=op.mult, op1=op.add,
                )
                nc.gpsimd.dma_start(out=ncscr[c : c + 1], in_=ncol[P - 1, 0:1])
                # posv = rowscan + rowoff for masked cells, >= BIG otherwise
                pos1 = cmpp.tile([P, P], f32, tag="pos1")
                nc.vector.tensor_scalar(
                    out=pos1[:], in0=rowscan_ps[:], scalar1=rowoffp[:, 0:1],
                    scalar2=None, op0=op.add,
                )
                posv = cmpp.tile([P, P], f32, tag="posv")
                nc.vector.scalar_tensor_tensor(
                    out=posv[:], in0=maskf[:], scalar=-BIG, in1=pos1[:],
                    op0=op.mult, op1=op.add,
                )
                # scatter cell ids to their slots (transposed orientation:
                # stationary = one-hot chunk, moving = a [128,2] rhs staged
                # per column with the partition-index / cc+1 constants), so
                # slot s accumulates (hi, lo) at chl[s%128, 2*(s//128)+{0,1}]
                chl = psB.tile([P, 2 * nchunk], f32, tag="chl")
                nc.vector.memset(chl[:], 0.0)

                def body_sc(iv):
                    oh = ohp.tile([P, cap], bf16, tag="ohsc")
                    nc.vector.tensor_scalar(
                        out=oh[:], in0=iotacap[:],
                        scalar1=posv[:, bass.ds(iv, 1)], scalar2=None,
                        op0=op.is_equal,
                    )
                    hl2 = ohp.tile([P, 2], bf16, tag="hl2")
                    nc.vector.tensor_copy(out=hl2[:, 0:1], in_=hrow_sb[:, bass.ds(iv, 1)])
                    nc.vector.tensor_copy(out=hl2[:, 1:2], in_=lrow_sb[:, bass.ds(iv, 1)])
                    for g in range(nchunk):
                        nc.tensor.matmul(
                            out=chl[:, 2 * g : 2 * g + 2],
                            lhsT=oh[:, g * P : (g + 1) * P],
                            rhs=hl2[:], start=False, stop=True,
                            skip_group_check=True,
                        )

                tc.For_i_unrolled(0, P, 1, body_sc, max_unroll=16)

                # cand id+1 per slot = 128*hi + lo; store slot-major and
                # broadcast back to all partitions
                chl_sb = cmpp.tile([P, 2 * nchunk], f32, tag="chl_sb")
                nc.vector.tensor_copy(out=chl_sb[:], in_=chl[:])
                chl3 = chl_sb[:].rearrange("p (g t) -> p t g", t=2)
                candT = cmpp.tile([P, nchunk], f32, tag="candT")
                nc.vector.scalar_tensor_tensor(
                    out=candT[:], in0=chl3[:, 0], scalar=128.0, in1=chl3[:, 1],
                    op0=op.mult, op1=op.add,
                )
                nc.gpsimd.dma_start(
                    out=candscr[c].rearrange("(g p) -> p g", p=P), in_=candT[:]
                )
                cand_f = cmpp.tile([P, cap], f32, tag="cand_f")
                cst = candscr.tensor if hasattr(candscr, "tensor") else candscr
                nc.gpsimd.dma_start(
                    out=cand_f[:],
                    in_=bass.AP(tensor=cst, offset=c * cap, ap=[[0, P], [1, cap]]),
                )
                candbc = cmpp.tile([P, cap], i16, tag="candbc")
                nc.vector.tensor_copy(out=candbc[:], in_=cand_f[:])

                # ---- phase 2: fine counts on candidate cells ----
                tc14p1 = cloudp.tile([P, Fl], f32, tag="tc14p1")
                nc.vector.tensor_scalar(
                    out=tc14p1[:], in0=rc[:], scalar1=1.0, scalar2=None, op0=op.add
                )
                rl = rawp.tile([P, Fl], u8, tag="rl")
                lsrc = lo6ps[c // nq][c % nq]
                nc.gpsimd.dma_start(out=rl[:], in_=lsrc.rearrange("(p f) -> p f", p=P))
                tlow6 = cloudp.tile([P, Fl], f32, tag="tlow6")
                nc.vector.tensor_copy(out=tlow6[:], in_=rl[:])

                hist = psD.tile([P, cap], f32, tag="hist")
                nc.vector.memset(hist[:], 0.0)

                def body2(iv):
                    memb = ohp.tile([P, cap], bf16, tag="memb")
                    loh = ohp.tile([P, 64], bf16, tag="loh")
                    nc.vector.tensor_scalar(
                        out=memb[:], in0=candbc[:],
                        scalar1=tc14p1[:, bass.ds(iv, 1)], scalar2=None,
                        op0=op.is_equal,
                    )
                    nc.vector.tensor_scalar(
                        out=loh[:], in0=iota64_bf[:],
                        scalar1=tlow6[:, bass.ds(iv, 1)], scalar2=None,
                        op0=op.is_equal,
                    )
                    for g in range(cap // 512):
                        sl = slice(g * 512, (g + 1) * 512)
                        nc.tensor.matmul(
                            out=hist[:64, sl], lhsT=loh[:], rhs=memb[:, sl],
                            start=False, stop=True, skip_group_check=True,
                        )

                tc.For_i_unrolled(0, Fl, 1, body2, max_unroll=16)

                # K = #(count >= 53): per-partition counts, then a DRAM
                # roundtrip to land the 64 partials on partition 0
                maskk = mkp.tile([P, cap], bf16, tag="maskk")
                nc.vector.tensor_scalar(
                    out=maskk[:64, :], in0=hist[:64, :],
                    scalar1=52.5, scalar2=None, op0=op.is_ge,
                )
                ks = mkp.tile([P, 1], f32, tag="ks")
                nc.vector.tensor_reduce(
                    out=ks[:64, :], in_=maskk[:64, :],
                    axis=mybir.AxisListType.X, op=op.add,
                )
                nc.gpsimd.dma_start(
                    out=kscr[c].rearrange("(p o) -> p o", o=1), in_=ks[:64, 0:1]
                )

            for c in range(n_clouds):
                kr = constp.tile([1, 64], f32, tag=f"kr{c}")
                nc.gpsimd.dma_start(
                    out=kr[:], in_=kscr[c].rearrange("(o f) -> o f", o=1)
                )
                nc.vector.tensor_reduce(
                    out=kv_sb[:1, c : c + 1], in_=kr[:],
                    axis=mybir.AxisListType.X, op=op.add,
                )

            # ncand values (still offset by BIG) -> row 0, cols [n, 2n)
            ncrow = constp.tile([1, n_clouds], f32)
            nc.gpsimd.dma_start(
                out=ncrow[:], in_=ncscr[:].rearrange("(o b) -> o b", o=1)
            )
            nc.vector.tensor_scalar(
                out=kv_sb[:1, n_clouds : 2 * n_clouds], in0=ncrow[:],
                scalar1=BIG, scalar2=None, op0=op.subtract,
            )
            nc.gpsimd.dma_start(out=kvals[:, :], in_=kv_sb[:])
    nc.compile()
    return nc


class _Runner:
    """jit(shard_map(bass_exec)) callable over 8 cores with device-resident
    inputs.  Mirrors concourse.bass2jax.run_bass_via_pjrt's lowering (the
    @via_axon target of bass_utils.run_bass_kernel_spmd), but accepts jax
    Arrays already placed on the devices so repeated launches don't re-ship
    inputs, and keeps the (never-donated, fully-overwritten) output
    parameter slots device-resident too."""

    def __init__(self, nc, n_cores=NCORES):
        import jax
        from concourse import bass2jax
        import concourse.mybir as mybir
        from jax.experimental.shard_map import shard_map
        from jax.sharding import Mesh, PartitionSpec, NamedSharding

        bass2jax.install_neuronx_cc_hook()
        assert not nc.dbg_callbacks if nc.dbg_addr is not None else True
        partition_name = (
            nc.partition_id_tensor.name if nc.partition_id_tensor else None
        )
        self.jax = jax
        self.n_cores = n_cores
        devices = jax.devices()[:n_cores]
        assert len(devices) == n_cores
        self.devices = devices
        self.mesh = Mesh(np.asarray(devices), ("core",))
        self.sharding = NamedSharding(self.mesh, PartitionSpec("core"))

        in_names, out_names, out_avals = [], [], []
        in_meta = {}
        for alloc in nc.m.functions[0].allocations:
            if not isinstance(alloc, mybir.MemoryLocationSet):
                continue
            name = alloc.memorylocations[0].name
            if alloc.kind == "ExternalInput":
                if name == partition_name:
                    continue
                in_names.append(name)
                in_meta[name] = (tuple(alloc.tensor_shape), mybir.dt.np(alloc.dtype))
            elif alloc.kind == "ExternalOutput":
                out_names.append(name)
                out_avals.append(
                    jax.core.ShapedArray(
                        tuple(alloc.tensor_shape), mybir.dt.np(alloc.dtype)
                    )
                )
        self.in_names, self.out_names = in_names, out_names
        self.in_meta = in_meta
        all_in = tuple(in_names) + tuple(out_names)
        if partition_name is not None:
            all_in = all_in + (partition_name,)

        def _body(*args):
            operands = list(args)
            if partition_name is not None:
                operands.append(bass2jax.partition_id_tensor())
            outs = bass2jax._bass_exec_p.bind(
                *operands,
                out_avals=tuple(out_avals),
                in_names=all_in,
                out_names=tuple(out_names),
                lowering_input_output_aliases=(),
                sim_require_finite=True,
                sim_require_nnan=True,
                nc=nc,
            )
            return tuple(outs)

        pspec = PartitionSpec("core")
        n_args = len(in_names) + len(out_names)
        self.fn = jax.jit(
            shard_map(
                _body,
                mesh=self.mesh,
                in_specs=(pspec,) * n_args,
                out_specs=(pspec,) * len(out_names),
                check_rep=False,
            ),
            keep_unused=True,
        )
        # persistent device-resident buffers for the output parameter slots
        # (never donated; the kernels fully overwrite every output element)
        self.out_dummies = [
            jax.device_put(
                np.zeros((n_cores * av.shape[0], *av.shape[1:]), av.dtype),
                self.sharding,
            )
            for av in out_avals
        ]
        self.extra = {}

    def __call__(self, arrays):
        args = []
        for name in self.in_names:
            if name in arrays:
                args.append(arrays[name])
            else:
                if name not in self.extra:
                    shape, dt = self.in_meta[name]
                    z = np.zeros((self.n_cores * shape[0], *shape[1:]), dt)
                    self.extra[name] = self.jax.device_put(z, self.sharding)
                args.append(self.extra[name])
        outs = self.fn(*args, *self.out_dummies)
        return dict(zip(self.out_names, outs))


def _state():
    if "st" in _cache:
        return _cache["st"]
    import jax
    import ml_dtypes

    ncf = build_fused(n_clouds=B // 4 // NCORES, nparams=1)
    runf = _Runner(ncf)
    # host-built constants, uploaded once and kept device-resident (each
    # core gets its own copy: global shape [8*128, 128])
    bf = ml_dtypes.bfloat16
    pidx = np.arange(P)
    tri = (pidx[:, None] < pidx[None, :]).astype(bf)
    ident = np.eye(P).astype(bf)
    hrow = np.broadcast_to(pidx[:, None], (P, P)).astype(bf)
    lrow = np.broadcast_to(pidx[None, :] + 1, (P, P)).astype(bf)
    consts = {
        name: jax.device_put(np.ascontiguousarray(np.tile(a, (NCORES, 1))), runf.sharding)
        for name, a in (("tri", tri), ("ident", ident), ("hrow", hrow), ("lrow", lrow))
    }
    BQ = B // 4
    st = {
        "jax": jax,
        "runf": runf,
        "consts": consts,
        "devices": runf.devices,
        "sharding": runf.sharding,
        # persistent host work buffers (avoid first-touch page faults on the
        # timed warm call)
        "t": np.empty((BQ, N, 2), np.float32),
        "sf": np.empty((BQ, N), np.float32),
        "frac": np.empty((BQ, N), np.float32),
        "intp": np.empty((BQ, N), np.float32),
        "c14": np.empty((B, N), np.uint16),
        "lo6": np.empty((B, N), np.uint8),
        # c14 = floor(s/64) = 16*qx + floor(qz/64): exact in f32 (value
        # < 2^14, qz/64 on a 2^-6 grid, f32 ulp at 2^14 is 2^-9), and the
        # truncating u16 cast is the floor
        "wc14": np.asarray([16.0, 0.015625], np.float32),
    }
    _cache["st"] = st
    return st


def _quant_quarter(st, pcd_q, qi):
    """Quantize one batch-quarter [B/4, N, 2] into the c14/lo6 plane slices.
    Exact: q = rint(1000*p) in f32 (matches jnp.round); with s = qx*1024+qz,
    c14 = s>>6 = 16*qx + (qz>>6) and lo6 = s&63 = qz&63 (1024 = 0 mod 64)."""
    BQ = B // 4
    sl = slice(qi * BQ, (qi + 1) * BQ)
    t = st["t"]
    np.multiply(pcd_q, np.float32(1000.0), out=t)
    np.rint(t, out=t)
    mn = t.min(axis=(1, 2))
    mx = t.max(axis=(1, 2))
    good = (mn >= 0) & (mx <= 1023)
    if not good.all():
        t[~good] = 0.0  # keep device indices in range; host recomputes these
    sf, frac, intp = st["sf"], st["frac"], st["intp"]
    np.dot(t.reshape(-1, 2), st["wc14"], out=sf.reshape(-1))
    # int part -> c14, frac*64 -> lo6 (both exact: validated exhaustively
    # over all (qx, qz) in [0,1023]^2)
    np.modf(sf, frac, intp)
    np.copyto(st["c14"][sl], intp, casting="unsafe")
    np.multiply(frac, np.float32(64.0), out=frac)
    np.copyto(st["lo6"][sl], frac, casting="unsafe")
    return good


def _host_exact(points):
    """Exact numpy replica of the reference for one cloud. [N,2] f32 -> [TOPK]."""
    q = np.round(np.float32(1000.0) * points.astype(np.float32))
    xi = (q[:, 0] - q[:, 0].min()).astype(np.int64)
    zi = (q[:, 1] - q[:, 1].min()).astype(np.int64)
    idx = xi * GZ + zi
    counts = np.bincount(idx, minlength=1024 * GZ).astype(np.float32)
    occ = counts / np.float32(points.shape[0]) > np.float32(0.0002)
    k = min(int(occ.sum()), TOPK)
    out = np.zeros((TOPK,), np.float32)
    out[:k] = 1.0
    return out


def kernel(pcd):
    import time

    t_start = time.time()
    pcd = np.ascontiguousarray(np.asarray(pcd), dtype=np.float32)
    assert pcd.shape == (B, N, 2), pcd.shape
    st = _state()
    jax = st["jax"]
    sharding = st["sharding"]
    _dbg("state ready", t_start)

    # pipeline: quantize batch-quarter q on the (single) CPU while a single
    # uploader thread streams finished quarters over the tunnel (sharded
    # device_put blocks until the transfer lands, so it must live off the
    # main thread) and then immediately dispatches that quarter's fused
    # launch (dispatch is async; exec overlaps the next quarter's upload).
    from concurrent.futures import ThreadPoolExecutor

    BQ = B // 4
    goods = [None] * 4
    quarters = pcd.reshape(4, BQ, N, 2)

    def _upload_and_launch(qi):
        sl = slice(qi * BQ, (qi + 1) * BQ)
        c = jax.device_put(st["c14"][sl], sharding)
        l = jax.device_put(st["lo6"][sl], sharding)
        r = st["runf"]({"c14p0": c, "lo6p0": l, **st["consts"]})
        return c, l, r

    with ThreadPoolExecutor(max_workers=1) as ex:
        futs = [None] * 4
        for qi in range(4):
            goods[qi] = _quant_quarter(st, quarters[qi], qi)
            futs[qi] = ex.submit(_upload_and_launch, qi)
        good = np.concatenate(goods)
        _dbg("quantize done, uploads in flight", t_start)
        results = [f.result() for f in futs]
    _dbg("uploads done, launches dispatched", t_start)

    CQ = BQ // NCORES  # clouds per core per quarter-launch
    out = np.zeros((B, TOPK, 1), np.float32)
    iota = np.arange(TOPK)
    for qi in range(4):
        kv = np.asarray(results[qi][2]["kvals"]).reshape(NCORES, 2 * CQ)
        for i in range(NCORES):
            for c in range(CQ):
                b = qi * BQ + CQ * i + c
                if good[b] and kv[i, CQ + c] <= CAND_CAP:
                    out[b, :, 0] = iota < kv[i, c]
                else:
                    out[b, :, 0] = _host_exact(pcd[b])
    _dbg("assembled", t_start)
    return out


# revision 41
# speedup vs baseline: 1.8742x; 1.1815x over previous
"""Trainium2 Bass kernel for nn_DeepMapping2D (histogram_binning).

Reference semantics: per cloud, quantize points to integer mm bins
(q = round_half_even(1000*p)), histogram into a 1024x1024 grid (shifted by
per-cloud coordinate minima), threshold counts (count/N > 2e-4 <=> count>=53),
sort the 0/1 occupancy descending, truncate to TOPK.  The sorted vector is K
ones then zeros, K = #bins with count >= 53.  Shifting by the minima is a
bijection on occupied bins, so K is shift-invariant and the device can work
on unshifted bin ids s = qx*1024 + qz (fine id, < 2^20).

Device algorithm (exact, two launches, all heavy counting on device):

Phase 1 (screen): per cloud, the exact 2^14-bin coarse histogram H14 over
c14 = s>>6, computed as a PSUM-matmul scatter: per column of 128 points,
build 128-wide one-hots of hi7=c14>>7 and lo7=c14&127 by comparing a
constant iota row against the point's value (DVE tensor_scalar is_equal with
a per-partition scalar), then accumulate onehot_hi^T @ onehot_lo into PSUM
(bf16 0/1 inputs are exact; fp32 accumulation).  H14, clamped to u8, goes
back to the host (1 MB total).

Host: candidate cells = {c14 : H14[c14] >= 53} (every fine bin with count
>= 53 lives in one, since H14 upper-bounds its 64 fine bins).  ~1.4k/cloud
for the rbg-generated inputs.  Padded with -1 to NCHUNK*128 int16.

Phase 2 (refine): per cloud, exact fine counts for every candidate cell:
per column, one membership one-hot against the candidate row (int16
candidates streamed at DVE 4x, compared against the point's c14 as the
per-partition scalar) and one 64-wide one-hot of low6 = s&63; NCHUNK
matmuls accumulate membership^T @ onehot_low6 into PSUM -> exact
[candidate, low6] fine counts.  Threshold >= 53, count via ones^T @ mask
matmul, giving K per cloud.  The host formats the final rows (K ones then
zeros) from the device-computed K values.

Transport optimization (the axon tunnel runs at ~35 MB/s, so bytes moved
dominate wall time): the host quantizes once into two integer planes,
c14 = s>>6 (uint16) and lo6 = s&63 (uint8) - 3 B/point = 50 MB instead of
the 8 B/point raw floats - and uploads them a single time.  Both phases
run through a jit(shard_map(bass_exec)) callable (the same primitive
bass_utils.run_bass_kernel_spmd lowers to under axon) against the SAME
device-resident plane arrays, so phase 2 re-reads them from device DRAM
instead of re-shipping 128 MB.  Quantization is pipelined per core-shard
with the uploads.

Host guards keep the kernel exact for arbitrary inputs: clouds with
coordinates outside [0, 1023] mm (or counts exceeding the candidate
capacity) fall back to an exact numpy recomputation of that cloud.

Sharding: data-parallel over batch: 64 clouds -> 8 cores x 8 clouds.
"""

import os
import numpy as np

B = 64
N = 262144
TOPK = 5120
NCORES = 8
CLOUDS_PER_CORE = B // NCORES
P = 128
F = N // P
GZ = 1024
NCHUNK = 12  # candidate capacity = NCHUNK*128 cells per cloud
CAND_CAP = NCHUNK * P
THRESH_COUNT = 53.0
C23 = 12582912.0  # 1.5 * 2^23

_cache = {}
_DEBUG = os.environ.get("KERNEL_DEBUG", "0") == "1"


def _dbg(msg, t0=None):
    if _DEBUG:
        import time

        if t0 is not None:
            print(f"[kernel] {msg}: {time.time()-t0:.3f}s", flush=True)
        else:
            print(f"[kernel] {msg}", flush=True)


def build_phase1(n_clouds=CLOUDS_PER_CORE, n_points=N, unroll=32):
    """Per-cloud exact 2^14-bin coarse histogram -> DRAM (u8, clamped)."""
    import concourse.bass as bass
    import concourse.mybir as mybir
    from concourse.tile import TileContext
    from concourse import bacc

    f32, bf16 = mybir.dt.float32, mybir.dt.bfloat16
    i32, u16, u8 = mybir.dt.int32, mybir.dt.uint16, mybir.dt.uint8
    op = mybir.AluOpType
    Fl = n_points // P

    nc = bacc.Bacc("TRN2", target_bir_lowering=False, debug=False)
    nq = n_clouds // 4
    c14ps = [
        nc.declare_dram_parameter(f"c14p{q}", [nq, n_points], u16, isOutput=False)
        for q in range(4)
    ]
    h14 = nc.declare_dram_parameter("h14", [n_clouds, P, P], u8, isOutput=True)

    with TileContext(nc) as tc:
        with (
            tc.tile_pool(name="const", bufs=1) as constp,
            tc.tile_pool(name="raw", bufs=2) as rawp,
            tc.tile_pool(name="chain", bufs=2) as chainp,
            tc.tile_pool(name="hilo", bufs=1) as hilop,
            tc.tile_pool(name="oh", bufs=8) as ohp,
            tc.tile_pool(name="hout", bufs=2) as houtp,
            tc.tile_pool(name="psum", bufs=1, space="PSUM") as psump,
        ):
            iota_i = constp.tile([P, P], i32)
            nc.gpsimd.iota(iota_i[:], pattern=[[1, P]], base=0, channel_multiplier=0)
            iota_bf = constp.tile([P, P], bf16)
            nc.vector.tensor_copy(out=iota_bf[:], in_=iota_i[:])

            this, tlos, hists = [], [], []
            for c in range(n_clouds):
                rc = rawp.tile([P, Fl], u16, tag="rc")
                src = c14ps[c // nq][c % nq]
                nc.gpsimd.dma_start(out=rc[:], in_=src.rearrange("(p f) -> p f", p=P))
                tc14 = chainp.tile([P, Fl], f32, tag="tc14")
                nc.vector.tensor_copy(out=tc14[:], in_=rc[:])
                # hi7 = floor(c14/128); lo7 = c14 - 128*hi7
                thif = chainp.tile([P, Fl], f32, tag="thif")
                nc.vector.tensor_scalar(
                    out=thif[:], in0=tc14[:], scalar1=0.0078125,
                    scalar2=0.49609375, op0=op.mult, op1=op.subtract,
                )
                thi = hilop.tile([P, Fl], f32, tag=f"thi{c}")
                nc.vector.tensor_scalar(
                    out=thi[:], in0=thif[:], scalar1=C23, scalar2=C23,
                    op0=op.add, op1=op.subtract,
                )
                tlo = hilop.tile([P, Fl], f32, tag=f"tlo{c}")
                nc.vector.scalar_tensor_tensor(
                    out=tlo[:], in0=thi[:], scalar=-128.0, in1=tc14[:],
                    op0=op.mult, op1=op.add,
                )
                this.append(thi)
                tlos.append(tlo)
                hist = psump.tile([P, P], f32, tag=f"hist{c}")
                nc.vector.memset(hist[:], 0.0)
                hists.append(hist)

            def body(iv):
                for c in range(n_clouds):
                    ohh = ohp.tile([P, P], bf16, tag="ohh")
                    ohl = ohp.tile([P, P], bf16, tag="ohl")
                    nc.vector.tensor_scalar(
                        out=ohh[:], in0=iota_bf[:],
                        scalar1=this[c][:, bass.ds(iv, 1)], scalar2=None,
                        op0=op.is_equal,
                    )
                    nc.vector.tensor_scalar(
                        out=ohl[:], in0=iota_bf[:],
                        scalar1=tlos[c][:, bass.ds(iv, 1)], scalar2=None,
                        op0=op.is_equal,
                    )
                    nc.tensor.matmul(
                        out=hists[c][:], lhsT=ohh[:], rhs=ohl[:],
                        start=False, stop=True, skip_group_check=True,
                    )

            tc.For_i_unrolled(0, Fl, 1, body, max_unroll=unroll)

            for c in range(n_clouds):
                hcl = houtp.tile([P, P], f32, tag="hcl")
                nc.vector.tensor_scalar(
                    out=hcl[:], in0=hists[c][:], scalar1=255.0, scalar2=None,
                    op0=op.min,
                )
                hu8 = houtp.tile([P, P], u8, tag="hu8")
                nc.vector.tensor_copy(out=hu8[:], in_=hcl[:])
                nc.gpsimd.dma_start(out=h14[c], in_=hu8[:])
    nc.compile()
    return nc


def build_phase2(n_clouds=CLOUDS_PER_CORE, n_points=N, nchunk=NCHUNK, unroll=16):
    """Exact [candidate,64] fine counts -> K per cloud."""
    import concourse.bass as bass
    import concourse.mybir as mybir
    from concourse.tile import TileContext
    from concourse import bacc

    f32, bf16 = mybir.dt.float32, mybir.dt.bfloat16
    i16, i32 = mybir.dt.int16, mybir.dt.int32
    u16, u8 = mybir.dt.uint16, mybir.dt.uint8
    op = mybir.AluOpType
    Fl = n_points // P
    cap = nchunk * P

    nc = bacc.Bacc("TRN2", target_bir_lowering=False, debug=False)
    nq = n_clouds // 4
    c14ps = [
        nc.declare_dram_parameter(f"c14p{q}", [nq, n_points], u16, isOutput=False)
        for q in range(4)
    ]
    lo6ps = [
        nc.declare_dram_parameter(f"lo6p{q}", [nq, n_points], u8, isOutput=False)
        for q in range(4)
    ]
    cands = nc.declare_dram_parameter("cands", [n_clouds, cap], i16, isOutput=False)
    kvals = nc.declare_dram_parameter("kvals", [1, n_clouds], f32, isOutput=True)

    with TileContext(nc) as tc:
        with (
            tc.tile_pool(name="const", bufs=1) as constp,
            tc.tile_pool(name="raw", bufs=2) as rawp,
            tc.tile_pool(name="cloud", bufs=2) as cloudp,
            tc.tile_pool(name="oh", bufs=8) as ohp,
            tc.tile_pool(name="mk", bufs=4) as mkp,
            tc.tile_pool(name="psum", bufs=1, space="PSUM") as psump,
            tc.tile_pool(name="kps", bufs=1, space="PSUM") as kpsp,
        ):
            iota64_i = constp.tile([P, 64], i32)
            nc.gpsimd.iota(iota64_i[:], pattern=[[1, 64]], base=0, channel_multiplier=0)
            iota64_bf = constp.tile([P, 64], bf16)
            nc.vector.tensor_copy(out=iota64_bf[:], in_=iota64_i[:])
            ones_bf = constp.tile([P, 1], bf16)
            nc.vector.memset(ones_bf[:], 1.0)
            kv_sb = constp.tile([1, n_clouds], f32)

            for c in range(n_clouds):
                rc = rawp.tile([P, Fl], u16, tag="rc")
                csrc = c14ps[c // nq][c % nq]
                nc.gpsimd.dma_start(out=rc[:], in_=csrc.rearrange("(p f) -> p f", p=P))
                tc14 = cloudp.tile([P, Fl], f32, tag="tc14")
                nc.vector.tensor_copy(out=tc14[:], in_=rc[:])
                rl = rawp.tile([P, Fl], u8, tag="rl")
                lsrc = lo6ps[c // nq][c % nq]
                nc.gpsimd.dma_start(out=rl[:], in_=lsrc.rearrange("(p f) -> p f", p=P))
                tlow6 = cloudp.tile([P, Fl], f32, tag="tlow6")
                nc.vector.tensor_copy(out=tlow6[:], in_=rl[:])

                # candidate row broadcast to all partitions
                candbc = cloudp.tile([P, cap], i16, tag="candbc")
                cand_src = bass.AP(
                    tensor=cands.tensor if hasattr(cands, "tensor") else cands,
                    offset=c * cap,
                    ap=[[0, P], [1, cap]],
                )
                nc.gpsimd.dma_start(out=candbc[:], in_=cand_src)

                hist = psump.tile([P, cap], f32, tag="hist")
                nc.vector.memset(hist[:], 0.0)

                def body(iv):
                    memb = ohp.tile([P, cap], bf16, tag="memb")
                    loh = ohp.tile([P, 64], bf16, tag="loh")
                    nc.vector.tensor_scalar(
                        out=memb[:], in0=candbc[:],
                        scalar1=tc14[:, bass.ds(iv, 1)], scalar2=None,
                        op0=op.is_equal,
                    )
                    nc.vector.tensor_scalar(
                        out=loh[:], in0=iota64_bf[:],
                        scalar1=tlow6[:, bass.ds(iv, 1)], scalar2=None,
                        op0=op.is_equal,
                    )
                    # transposed accumulation: hist[w, cand] += loh^T @ memb,
                    # 512-wide moving slices so the 64-wide stationary loh is
                    # shared and PE streams at full width
                    for g in range(cap // 512):
                        nc.tensor.matmul(
                            out=hist[:64, g * 512 : (g + 1) * 512],
                            lhsT=loh[:],
                            rhs=memb[:, g * 512 : (g + 1) * 512],
                            start=False, stop=True, skip_group_check=True,
                        )

                tc.For_i_unrolled(0, Fl, 1, body, max_unroll=unroll)

                # K = sum over candidates/low6 of [count >= 53]
                kps = kpsp.tile([1, cap], f32, tag="kps")
                for g in range(cap // 512):
                    mask = mkp.tile([P, 512], bf16, tag="mask")
                    nc.vector.tensor_scalar(
                        out=mask[:64, :], in0=hist[:64, g * 512 : (g + 1) * 512],
                        scalar1=52.5, scalar2=None, op0=op.is_ge,
                    )
                    nc.tensor.matmul(
                        out=kps[:1, g * 512 : (g + 1) * 512],
                        lhsT=ones_bf[:64, :], rhs=mask[:64, :],
                        start=True, stop=True,
                    )
                nc.vector.tensor_reduce(
                    out=kv_sb[:1, c : c + 1], in_=kps[:],
                    axis=mybir.AxisListType.X, op=op.add,
                )

            nc.gpsimd.dma_start(out=kvals[:, :], in_=kv_sb[:])
    nc.compile()
    return nc


def build_fused(n_clouds=CLOUDS_PER_CORE, n_points=N, nchunk=NCHUNK, nparams=4):
    """Single-launch kernel: coarse histogram -> on-device candidate
    compaction (threshold mask -> exclusive prefix scan via triangular
    matmuls -> position-one-hot scatter of cell ids) -> fine refine -> K.
    Outputs kvals [1, 2*n_clouds]: cols [0,n) = K, cols [n,2n) = ncand
    (host falls back if ncand > capacity)."""
    import concourse.bass as bass
    import concourse.mybir as mybir
    from concourse.tile import TileContext
    from concourse import bacc

    f32, bf16 = mybir.dt.float32, mybir.dt.bfloat16
    i16, i32 = mybir.dt.int16, mybir.dt.int32
    u16, u8 = mybir.dt.uint16, mybir.dt.uint8
    op = mybir.AluOpType
    Fl = n_points // P
    cap = nchunk * P
    BIG = 16384.0

    nc = bacc.Bacc("TRN2", target_bir_lowering=False, debug=False)
    nq = n_clouds // nparams
    c14ps = [
        nc.declare_dram_parameter(f"c14p{q}", [nq, n_points], u16, isOutput=False)
        for q in range(nparams)
    ]
    lo6ps = [
        nc.declare_dram_parameter(f"lo6p{q}", [nq, n_points], u8, isOutput=False)
        for q in range(nparams)
    ]
    # constants built on the host, uploaded once and kept device-resident:
    # tri[p,j] = 1 if p<j (strictly-upper triangle), ident = I128,
    # hrow[:,j] = partition index, lrow[p,j] = j+1
    tri = nc.declare_dram_parameter("tri", [P, P], bf16, isOutput=False)
    ident = nc.declare_dram_parameter("ident", [P, P], bf16, isOutput=False)
    hrow = nc.declare_dram_parameter("hrow", [P, P], bf16, isOutput=False)
    lrow = nc.declare_dram_parameter("lrow", [P, P], bf16, isOutput=False)
    kvals = nc.declare_dram_parameter("kvals", [1, 2 * n_clouds], f32, isOutput=True)
    candscr = nc.dram_tensor("candscr", [n_clouds, cap], f32)
    ncscr = nc.dram_tensor("ncscr", [n_clouds], f32)
    kscr = nc.dram_tensor("kscr", [n_clouds, 64], f32)

    with TileContext(nc) as tc:
        with (
            tc.tile_pool(name="const", bufs=1) as constp,
            tc.tile_pool(name="raw", bufs=2) as rawp,
            tc.tile_pool(name="cloud", bufs=2) as cloudp,
            tc.tile_pool(name="cmp", bufs=1) as cmpp,
            tc.tile_pool(name="oh", bufs=8) as ohp,
            tc.tile_pool(name="mk", bufs=4) as mkp,
            tc.tile_pool(name="psA", bufs=1, space="PSUM") as psA,
            tc.tile_pool(name="psB", bufs=1, space="PSUM") as psB,
            tc.tile_pool(name="psC", bufs=1, space="PSUM") as psC,
            tc.tile_pool(name="psD", bufs=1, space="PSUM") as psD,
        ):
            iota_i = constp.tile([P, P], i32)
            nc.gpsimd.iota(iota_i[:], pattern=[[1, P]], base=0, channel_multiplier=0)
            iota_bf = constp.tile([P, P], bf16)
            nc.vector.tensor_copy(out=iota_bf[:], in_=iota_i[:])
            iota64_i = constp.tile([P, 64], i32)
            nc.gpsimd.iota(iota64_i[:], pattern=[[1, 64]], base=0, channel_multiplier=0)
            iota64_bf = constp.tile([P, 64], bf16)
            nc.vector.tensor_copy(out=iota64_bf[:], in_=iota64_i[:])
            iotacap_i32 = constp.tile([P, cap], i32)
            nc.gpsimd.iota(
                iotacap_i32[:], pattern=[[1, cap]], base=0, channel_multiplier=0
            )
            iotacap = constp.tile([P, cap], i16)
            nc.vector.tensor_copy(out=iotacap[:], in_=iotacap_i32[:])
            ones_bf = constp.tile([P, 1], bf16)
            nc.vector.memset(ones_bf[:], 1.0)
            tri_sb = constp.tile([P, P], bf16)
            nc.gpsimd.dma_start(out=tri_sb[:], in_=tri[:, :])
            ident_sb = constp.tile([P, P], bf16)
            nc.gpsimd.dma_start(out=ident_sb[:], in_=ident[:, :])
            hrow_sb = constp.tile([P, P], bf16)
            nc.gpsimd.dma_start(out=hrow_sb[:], in_=hrow[:, :])
            lrow_sb = constp.tile([P, P], bf16)
            nc.gpsimd.dma_start(out=lrow_sb[:], in_=lrow[:, :])
            kv_sb = constp.tile([1, 2 * n_clouds], f32)

            for c in range(n_clouds):
                # ---- phase 1: coarse 2^14 histogram of c14 ----
                rc = rawp.tile([P, Fl], u16, tag="rc")
                csrc = c14ps[c // nq][c % nq]
                nc.gpsimd.dma_start(out=rc[:], in_=csrc.rearrange("(p f) -> p f", p=P))
                tc14 = cloudp.tile([P, Fl], f32, tag="tc14")
                nc.vector.tensor_copy(out=tc14[:], in_=rc[:])
                thif = cloudp.tile([P, Fl], f32, tag="thif")
                nc.vector.tensor_scalar(
                    out=thif[:], in0=tc14[:], scalar1=0.0078125,
                    scalar2=0.49609375, op0=op.mult, op1=op.subtract,
                )
                thi = cloudp.tile([P, Fl], f32, tag="thi")
                nc.vector.tensor_scalar(
                    out=thi[:], in0=thif[:], scalar1=C23, scalar2=C23,
                    op0=op.add, op1=op.subtract,
                )
                tlo = cloudp.tile([P, Fl], f32, tag="tlo")
                nc.vector.scalar_tensor_tensor(
                    out=tlo[:], in0=thi[:], scalar=-128.0, in1=tc14[:],
                    op0=op.mult, op1=op.add,
                )
                h14 = psA.tile([P, P], f32, tag="ps128")
                nc.vector.memset(h14[:], 0.0)

                def body1(iv):
                    ohh = ohp.tile([P, P], bf16, tag="ohh")
                    ohl = ohp.tile([P, P], bf16, tag="ohl")
                    nc.vector.tensor_scalar(
                        out=ohh[:], in0=iota_bf[:],
                        scalar1=thi[:, bass.ds(iv, 1)], scalar2=None,
                        op0=op.is_equal,
                    )
                    nc.vector.tensor_scalar(
                        out=ohl[:], in0=iota_bf[:],
                        scalar1=tlo[:, bass.ds(iv, 1)], scalar2=None,
                        op0=op.is_equal,
                    )
                    nc.tensor.matmul(
                        out=h14[:], lhsT=ohh[:], rhs=ohl[:],
                        start=False, stop=True, skip_group_check=True,
                    )

                tc.For_i_unrolled(0, Fl, 1, body1, max_unroll=32)

                # ---- candidate compaction ----
                mask_bf = cmpp.tile([P, P], bf16, tag="mask_bf")
                nc.vector.tensor_scalar(
                    out=mask_bf[:], in0=h14[:], scalar1=52.5, scalar2=None,
                    op0=op.is_ge,
                )
                maskf = cmpp.tile([P, P], f32, tag="maskf")
                nc.vector.tensor_copy(out=maskf[:], in_=mask_bf[:])
                rowsum = cmpp.tile([P, 1], f32, tag="rowsum")
                nc.vector.tensor_reduce(
                    out=rowsum[:], in_=maskf[:], axis=mybir.AxisListType.X, op=op.add
                )
                rowsum_bf = cmpp.tile([P, 1], bf16, tag="rowsum_bf")
                nc.vector.tensor_copy(out=rowsum_bf[:], in_=rowsum[:])
                # mT = mask^T (PE transpose), then row-exclusive scan and
                # row-offset prefix via the strict upper-triangular ones
                mT_ps = psB.tile([P, P], bf16, tag="mT_ps")
                nc.tensor.transpose(mT_ps[:], mask_bf[:], ident_sb[:])
                mT_bf = cmpp.tile([P, P], bf16, tag="mT_bf")
                nc.vector.tensor_copy(out=mT_bf[:], in_=mT_ps[:])
                rowscan_ps = psA.tile([P, P], f32, tag="ps128")
                nc.tensor.matmul(
                    out=rowscan_ps[:], lhsT=mT_bf[:], rhs=tri_sb[:],
                    start=True, stop=True,
                )
                rowoff_ps = psB.tile([P, 1], f32, tag="rowoff")
                nc.tensor.matmul(
                    out=rowoff_ps[:], lhsT=tri_sb[:], rhs=rowsum_bf[:],
                    start=True, stop=True,
                )
                rowoffp = cmpp.tile([P, 1], f32, tag="rowoffp")
                nc.vector.tensor_scalar(
                    out=rowoffp[:], in0=rowoff_ps[:], scalar1=BIG, scalar2=None,
                    op0=op.add,
                )
                # ncand (inclusive prefix at the last row) -> DRAM scratch
                ncol = cmpp.tile([P, 1], f32, tag="ncol")
                nc.vector.scalar_tensor_tensor(
                    out=ncol[:], in0=rowoffp[:], scalar=1.0, in1=rowsum[:],
                    op0=op.mult, op1=op.add,
                )
                nc.gpsimd.dma_start(out=ncscr[c : c + 1], in_=ncol[P - 1, 0:1])
                # posv = rowscan + rowoff for masked cells, >= BIG otherwise
                pos1 = cmpp.tile([P, P], f32, tag="pos1")
                nc.vector.tensor_scalar(
                    out=pos1[:], in0=rowscan_ps[:], scalar1=rowoffp[:, 0:1],
                    scalar2=None, op0=op.add,
                )
                posv = cmpp.tile([P, P], f32, tag="posv")
                nc.vector.scalar_tensor_tensor(
                    out=posv[:], in0=maskf[:], scalar=-BIG, in1=pos1[:],
                    op0=op.mult, op1=op.add,
                )
                # scatter cell ids to their slots (transposed orientation:
                # stationary = one-hot chunk, moving = a [128,2] rhs staged
                # per column with the partition-index / cc+1 constants), so
                # slot s accumulates (hi, lo) at chl[s%128, 2*(s//128)+{0,1}]
                chl = psB.tile([P, 2 * nchunk], f32, tag="chl")
                nc.vector.memset(chl[:], 0.0)

                def body_sc(iv):
                    oh = ohp.tile([P, cap], bf16, tag="ohsc")
                    nc.vector.tensor_scalar(
                        out=oh[:], in0=iotacap[:],
                        scalar1=posv[:, bass.ds(iv, 1)], scalar2=None,
                        op0=op.is_equal,
                    )
                    hl2 = ohp.tile([P, 2], bf16, tag="hl2")
                    nc.vector.tensor_copy(out=hl2[:, 0:1], in_=hrow_sb[:, bass.ds(iv, 1)])
                    nc.vector.tensor_copy(out=hl2[:, 1:2], in_=lrow_sb[:, bass.ds(iv, 1)])
                    for g in range(nchunk):
                        nc.tensor.matmul(
                            out=chl[:, 2 * g : 2 * g + 2],
                            lhsT=oh[:, g * P : (g + 1) * P],
                            rhs=hl2[:], start=False, stop=True,
                            skip_group_check=True,
                        )

                tc.For_i_unrolled(0, P, 1, body_sc, max_unroll=16)

                # cand id+1 per slot = 128*hi + lo; store slot-major and
                # broadcast back to all partitions
                chl_sb = cmpp.tile([P, 2 * nchunk], f32, tag="chl_sb")
                nc.vector.tensor_copy(out=chl_sb[:], in_=chl[:])
                chl3 = chl_sb[:].rearrange("p (g t) -> p t g", t=2)
                candT = cmpp.tile([P, nchunk], f32, tag="candT")
                nc.vector.scalar_tensor_tensor(
                    out=candT[:], in0=chl3[:, 0], scalar=128.0, in1=chl3[:, 1],
                    op0=op.mult, op1=op.add,
                )
                nc.gpsimd.dma_start(
                    out=candscr[c].rearrange("(g p) -> p g", p=P), in_=candT[:]
                )
                cand_f = cmpp.tile([P, cap], f32, tag="cand_f")
                cst = candscr.tensor if hasattr(candscr, "tensor") else candscr
                nc.gpsimd.dma_start(
                    out=cand_f[:],
                    in_=bass.AP(tensor=cst, offset=c * cap, ap=[[0, P], [1, cap]]),
                )
                candbc = cmpp.tile([P, cap], i16, tag="candbc")
                nc.vector.tensor_copy(out=candbc[:], in_=cand_f[:])

                # ---- phase 2: fine counts on candidate cells ----
                tc14p1 = cloudp.tile([P, Fl], f32, tag="tc14p1")
                nc.vector.tensor_scalar(
                    out=tc14p1[:], in0=rc[:], scalar1=1.0, scalar2=None, op0=op.add
                )
                rl = rawp.tile([P, Fl], u8, tag="rl")
                lsrc = lo6ps[c // nq][c % nq]
                nc.gpsimd.dma_start(out=rl[:], in_=lsrc.rearrange("(p f) -> p f", p=P))
                tlow6 = cloudp.tile([P, Fl], f32, tag="tlow6")
                nc.vector.tensor_copy(out=tlow6[:], in_=rl[:])

                hist = psD.tile([P, cap], f32, tag="hist")
                nc.vector.memset(hist[:], 0.0)

                def body2(iv):
                    memb = ohp.tile([P, cap], bf16, tag="memb")
                    loh = ohp.tile([P, 64], bf16, tag="loh")
                    nc.vector.tensor_scalar(
                        out=memb[:], in0=candbc[:],
                        scalar1=tc14p1[:, bass.ds(iv, 1)], scalar2=None,
                        op0=op.is_equal,
                    )
                    nc.vector.tensor_scalar(
                        out=loh[:], in0=iota64_bf[:],
                        scalar1=tlow6[:, bass.ds(iv, 1)], scalar2=None,
                        op0=op.is_equal,
                    )
                    for g in range(cap // 512):
                        sl = slice(g * 512, (g + 1) * 512)
                        nc.tensor.matmul(
                            out=hist[:64, sl], lhsT=loh[:], rhs=memb[:, sl],
                            start=False, stop=True, skip_group_check=True,
                        )

                tc.For_i_unrolled(0, Fl, 1, body2, max_unroll=16)

                # K = #(count >= 53): per-partition counts, then a DRAM
                # roundtrip to land the 64 partials on partition 0
                maskk = mkp.tile([P, cap], bf16, tag="maskk")
                nc.vector.tensor_scalar(
                    out=maskk[:64, :], in0=hist[:64, :],
                    scalar1=52.5, scalar2=None, op0=op.is_ge,
                )
                ks = mkp.tile([P, 1], f32, tag="ks")
                nc.vector.tensor_reduce(
                    out=ks[:64, :], in_=maskk[:64, :],
                    axis=mybir.AxisListType.X, op=op.add,
                )
                nc.gpsimd.dma_start(
                    out=kscr[c].rearrange("(p o) -> p o", o=1), in_=ks[:64, 0:1]
                )

            for c in range(n_clouds):
                kr = constp.tile([1, 64], f32, tag=f"kr{c}")
                nc.gpsimd.dma_start(
                    out=kr[:], in_=kscr[c].rearrange("(o f) -> o f", o=1)
                )
                nc.vector.tensor_reduce(
                    out=kv_sb[:1, c : c + 1], in_=kr[:],
                    axis=mybir.AxisListType.X, op=op.add,
                )

            # ncand values (still offset by BIG) -> row 0, cols [n, 2n)
            ncrow = constp.tile([1, n_clouds], f32)
            nc.gpsimd.dma_start(
                out=ncrow[:], in_=ncscr[:].rearrange("(o b) -> o b", o=1)
            )
            nc.vector.tensor_scalar(
                out=kv_sb[:1, n_clouds : 2 * n_clouds], in0=ncrow[:],
                scalar1=BIG, scalar2=None, op0=op.subtract,
            )
            nc.gpsimd.dma_start(out=kvals[:, :], in_=kv_sb[:])
    nc.compile()
    return nc


class _Runner:
    """jit(shard_map(bass_exec)) callable over 8 cores with device-resident
    inputs.  Mirrors concourse.bass2jax.run_bass_via_pjrt's lowering (the
    @via_axon target of bass_utils.run_bass_kernel_spmd), but accepts jax
    Arrays already placed on the devices so repeated launches don't re-ship
    inputs, and keeps the (never-donated, fully-overwritten) output
    parameter slots device-resident too."""

    def __init__(self, nc, n_cores=NCORES):
        import jax
        from concourse import bass2jax
        import concourse.mybir as mybir
        from jax.experimental.shard_map import shard_map
        from jax.sharding import Mesh, PartitionSpec, NamedSharding

        bass2jax.install_neuronx_cc_hook()
        assert not nc.dbg_callbacks if nc.dbg_addr is not None else True
        partition_name = (
            nc.partition_id_tensor.name if nc.partition_id_tensor else None
        )
        self.jax = jax
        self.n_cores = n_cores
        devices = jax.devices()[:n_cores]
        assert len(devices) == n_cores
        self.devices = devices
        self.mesh = Mesh(np.asarray(devices), ("core",))
        self.sharding = NamedSharding(self.mesh, PartitionSpec("core"))

        in_names, out_names, out_avals = [], [], []
        in_meta = {}
        for alloc in nc.m.functions[0].allocations:
            if not isinstance(alloc, mybir.MemoryLocationSet):
                continue
            name = alloc.memorylocations[0].name
            if alloc.kind == "ExternalInput":
                if name == partition_name:
                    continue
                in_names.append(name)
                in_meta[name] = (tuple(alloc.tensor_shape), mybir.dt.np(alloc.dtype))
            elif alloc.kind == "ExternalOutput":
                out_names.append(name)
                out_avals.append(
                    jax.core.ShapedArray(
                        tuple(alloc.tensor_shape), mybir.dt.np(alloc.dtype)
                    )
                )
        self.in_names, self.out_names = in_names, out_names
        self.in_meta = in_meta
        all_in = tuple(in_names) + tuple(out_names)
        if partition_name is not None:
            all_in = all_in + (partition_name,)

        def _body(*args):
            operands = list(args)
            if partition_name is not None:
                operands.append(bass2jax.partition_id_tensor())
            outs = bass2jax._bass_exec_p.bind(
                *operands,
                out_avals=tuple(out_avals),
                in_names=all_in,
                out_names=tuple(out_names),
                lowering_input_output_aliases=(),
                sim_require_finite=True,
                sim_require_nnan=True,
                nc=nc,
            )
            return tuple(outs)

        pspec = PartitionSpec("core")
        n_args = len(in_names) + len(out_names)
        self.fn = jax.jit(
            shard_map(
                _body,
                mesh=self.mesh,
                in_specs=(pspec,) * n_args,
                out_specs=(pspec,) * len(out_names),
                check_rep=False,
            ),
            keep_unused=True,
        )
        # persistent device-resident buffers for the output parameter slots
        # (never donated; the kernels fully overwrite every output element)
        self.out_dummies = [
            jax.device_put(
                np.zeros((n_cores * av.shape[0], *av.shape[1:]), av.dtype),
                self.sharding,
            )
            for av in out_avals
        ]
        self.extra = {}

    def __call__(self, arrays):
        args = []
        for name in self.in_names:
            if name in arrays:
                args.append(arrays[name])
            else:
                if name not in self.extra:
                    shape, dt = self.in_meta[name]
                    z = np.zeros((self.n_cores * shape[0], *shape[1:]), dt)
                    self.extra[name] = self.jax.device_put(z, self.sharding)
                args.append(self.extra[name])
        outs = self.fn(*args, *self.out_dummies)
        return dict(zip(self.out_names, outs))


NLAUNCH = int(os.environ.get("KERNEL_NLAUNCH", "1"))


def _state():
    if "st" in _cache:
        return _cache["st"]
    import jax
    import ml_dtypes

    nq_per_launch = 4 // NLAUNCH
    ncf = build_fused(
        n_clouds=CLOUDS_PER_CORE // NLAUNCH, nparams=nq_per_launch
    )
    runf = _Runner(ncf)
    # host-built constants, uploaded once and kept device-resident (each
    # core gets its own copy: global shape [8*128, 128])
    bf = ml_dtypes.bfloat16
    pidx = np.arange(P)
    tri = (pidx[:, None] < pidx[None, :]).astype(bf)
    ident = np.eye(P).astype(bf)
    hrow = np.broadcast_to(pidx[:, None], (P, P)).astype(bf)
    lrow = np.broadcast_to(pidx[None, :] + 1, (P, P)).astype(bf)
    consts = {
        name: jax.device_put(np.ascontiguousarray(np.tile(a, (NCORES, 1))), runf.sharding)
        for name, a in (("tri", tri), ("ident", ident), ("hrow", hrow), ("lrow", lrow))
    }
    BQ = B // 4
    st = {
        "jax": jax,
        "runf": runf,
        "consts": consts,
        "devices": runf.devices,
        "sharding": runf.sharding,
        # persistent host work buffers (avoid first-touch page faults on the
        # timed warm call)
        "t": np.empty((BQ, N, 2), np.float32),
        "sf": np.empty((BQ, N), np.float32),
        "frac": np.empty((BQ, N), np.float32),
        "intp": np.empty((BQ, N), np.float32),
        "c14": np.empty((B, N), np.uint16),
        "lo6": np.empty((B, N), np.uint8),
        # c14 = floor(s/64) = 16*qx + floor(qz/64): exact in f32 (value
        # < 2^14, qz/64 on a 2^-6 grid, f32 ulp at 2^14 is 2^-9), and the
        # truncating u16 cast is the floor
        "wc14": np.asarray([16.0, 0.015625], np.float32),
    }
    _cache["st"] = st
    return st


def _quant_quarter(st, pcd_q, qi):
    """Quantize one batch-quarter [B/4, N, 2] into the c14/lo6 plane slices.
    Exact: q = rint(1000*p) in f32 (matches jnp.round); with s = qx*1024+qz,
    c14 = s>>6 = 16*qx + (qz>>6) and lo6 = s&63 = qz&63 (1024 = 0 mod 64)."""
    BQ = B // 4
    sl = slice(qi * BQ, (qi + 1) * BQ)
    t = st["t"]
    np.multiply(pcd_q, np.float32(1000.0), out=t)
    np.rint(t, out=t)
    mn = t.min(axis=(1, 2))
    mx = t.max(axis=(1, 2))
    good = (mn >= 0) & (mx <= 1023)
    if not good.all():
        t[~good] = 0.0  # keep device indices in range; host recomputes these
    sf, frac, intp = st["sf"], st["frac"], st["intp"]
    np.dot(t.reshape(-1, 2), st["wc14"], out=sf.reshape(-1))
    # int part -> c14, frac*64 -> lo6 (both exact: validated exhaustively
    # over all (qx, qz) in [0,1023]^2)
    np.modf(sf, frac, intp)
    np.copyto(st["c14"][sl], intp, casting="unsafe")
    np.multiply(frac, np.float32(64.0), out=frac)
    np.copyto(st["lo6"][sl], frac, casting="unsafe")
    return good


def _host_exact(points):
    """Exact numpy replica of the reference for one cloud. [N,2] f32 -> [TOPK]."""
    q = np.round(np.float32(1000.0) * points.astype(np.float32))
    xi = (q[:, 0] - q[:, 0].min()).astype(np.int64)
    zi = (q[:, 1] - q[:, 1].min()).astype(np.int64)
    idx = xi * GZ + zi
    counts = np.bincount(idx, minlength=1024 * GZ).astype(np.float32)
    occ = counts / np.float32(points.shape[0]) > np.float32(0.0002)
    k = min(int(occ.sum()), TOPK)
    out = np.zeros((TOPK,), np.float32)
    out[:k] = 1.0
    return out


def kernel(pcd):
    import time

    t_start = time.time()
    pcd = np.ascontiguousarray(np.asarray(pcd), dtype=np.float32)
    assert pcd.shape == (B, N, 2), pcd.shape
    st = _state()
    jax = st["jax"]
    sharding = st["sharding"]
    _dbg("state ready", t_start)

    # pipeline: quantize batch-quarter q on the (single) CPU while a single
    # uploader thread streams finished quarters over the tunnel (sharded
    # device_put blocks until the transfer lands, so it must live off the
    # main thread) and then immediately dispatches that quarter's fused
    # launch (dispatch is async; exec overlaps the next quarter's upload).
    from concurrent.futures import ThreadPoolExecutor

    BQ = B // 4
    goods = [None] * 4
    quarters = pcd.reshape(4, BQ, N, 2)

    nqpl = 4 // NLAUNCH  # quarters per launch
    planes = [[None, None] for _ in range(4)]  # device arrays per quarter

    def _upload(qi):
        sl = slice(qi * BQ, (qi + 1) * BQ)
        planes[qi][0] = jax.device_put(st["c14"][sl], sharding)
        planes[qi][1] = jax.device_put(st["lo6"][sl], sharding)
        if qi % nqpl == nqpl - 1:  # last quarter of a launch group
            li = qi // nqpl
            args = {}
            for k in range(nqpl):
                args[f"c14p{k}"] = planes[li * nqpl + k][0]
                args[f"lo6p{k}"] = planes[li * nqpl + k][1]
            args.update(st["consts"])
            return st["runf"](args)
        return None

    with ThreadPoolExecutor(max_workers=1) as ex:
        futs = [None] * 4
        for qi in range(4):
            goods[qi] = _quant_quarter(st, quarters[qi], qi)
            futs[qi] = ex.submit(_upload, qi)
        good = np.concatenate(goods)
        _dbg("quantize done, uploads in flight", t_start)
        results = [f.result() for f in futs]
    _dbg("uploads done, launches dispatched", t_start)

    launches = [r for r in results if r is not None]
    CPL = CLOUDS_PER_CORE // NLAUNCH  # clouds per core per launch
    out = np.zeros((B, TOPK, 1), np.float32)
    iota = np.arange(TOPK)
    for li in range(NLAUNCH):
        kv = np.asarray(launches[li]["kvals"]).reshape(NCORES, 2 * CPL)
        for i in range(NCORES):
            for c in range(CPL):
                q = li * nqpl + c // 2
                b = 16 * q + 2 * i + (c % 2)
                if good[b] and kv[i, CPL + c] <= CAND_CAP:
                    out[b, :, 0] = iota < kv[i, c]
                else:
                    out[b, :, 0] = _host_exact(pcd[b])
    _dbg("assembled", t_start)
    return out
